# revision 1
# baseline (speedup 1.0000x reference)
"""Trainium2 Bass kernel for nn_DifferentiableParticleFilter (N=8192, 8 cores).

Sharding: the (N,N) soft-resample matrix is sharded by output rows (1024 per
core); the per-particle network + state (N,49) is computed replicated on each
core.  Host pre-transposes each u_gumbel shard so the contraction axis lands
on SBUF partitions.

Algebra used on device (tau = 0.5):
    exp(g/tau) = 1/v^2 with v = -log(u+1e-10)+1e-10,
    softmax row-normalizer obtained from the same matmul via a w-column,
    log-weights folded into the state rows: state_w[j] = w_j*[state_j | 1],
    w_j = exp(2*clamp(lw_j - max lw, -30, 0)).
Big-tensor pipeline per tile: DMA -> Ln -> Square(-t+eps) -> 1/x -> matmul.
"""

import numpy as np

import concourse.bass as bass
import concourse.bass_isa as bass_isa
import concourse.tile as tile
from concourse import bacc
from concourse import library_config, mybir
from concourse.bass_utils import run_bass_kernel_spmd

F32 = mybir.dt.float32
AF = mybir.ActivationFunctionType
ALU = mybir.AluOpType
AX = mybir.AxisListType

K_ACT = 5
EPS = 1.0e-10
LWCLAMP = -30.0
C_LL = float(np.log(2.0) - 0.5 * np.log(2.0 * np.pi))
INV_SQRT2 = float(1.0 / np.sqrt(2.0))

# one packed [128, C] parameter blob -> one DMA, one semaphore lane.
# (name, n_partitions, n_cols); offsets are cumulative in this order.
def _param_spec(JT):
    return [
        ("ident", 128, 128), ("lhsT_E1", 15, 33), ("lhsT_rt1", 16, 32),
        ("brow_rt1", 1, 32), ("lhsT_nlog", 47, 15), ("brow_nlog", 1, 15),
        ("lhsT_d1", 48, 64), ("brow_d1", 1, 64), ("lhsT_d2", 65, 32),
        ("lhsT_d3", 33, 4), ("lhsT_g", 48, 32), ("brow_g", 1, 32),
        ("lhsT_c", 48, 32), ("brow_c", 1, 32), ("lhsT_a1", 65, 16),
        ("lhsT_a2", 16, 1), ("brow_a2", 1, 1), ("h_col", 65, 1),
        ("log_obs5", 5, 1), ("logR0", 1, 1), ("obs11", 1, 1),
        ("rh_p", 128, JT), ("rlow_p", 128, JT), ("eh_p", 128, JT),
        ("el_p", 128, JT), ("lw0_p", 128, JT),
    ]


# ---------------------------------------------------------------------------
# device program (SPMD - one program, per-core inputs differ)
# ---------------------------------------------------------------------------

def build_program(n_particles, rows_per_core, sim_compat=False):
    N = int(n_particles)
    R = int(rows_per_core)
    JT = N // 128                 # j-tiles (contraction tiles of 128 particles)
    CH = min(1024, N)             # phase-A free chunk
    NQ = N // CH
    BW = min(512, CH)             # matmul moving width (phase A)
    G = min(8, JT)                # j-tiles per big-loop super tile
    SUP = JT // G
    MB = min(512, R)              # big-matmul moving width
    NB = R // MB
    OW = min(128, R)              # output transpose width
    OB = R // OW

    nc = bacc.Bacc("TRN2", target_bir_lowering=False, debug=False)
    ERF = AF.Tanh if sim_compat else AF.Erf

    def par(name, shape, out=False):
        return nc.declare_dram_parameter(name, list(shape), F32, isOutput=out)

    spec = _param_spec(JT)
    CP = sum(m for _, _, m in spec)
    d_uT = par("uT", (N, R))
    d_zT = par("zT", (32, N))
    d_logT = par("logitsT", (15, N))
    d_params = par("params", (128, CP))
    d_y = par("y", (R, 49), out=True)

    with tile.TileContext(nc) as tc:
        # ---- persistent tiles (single-tile pools) -------------------------
        _keep = []      # hold the free-callbacks so pools aren't GC-released

        def sm(shape, name):
            t, free = tc.tile(list(shape), F32, name=name)
            _keep.append(free)
            return t

        def smload(dram, shape, name):
            t = sm(shape, name)
            nc.sync.dma_start(t[:], dram[:])
            return t

        P = smload(d_params, (128, CP), "P")
        _views = {}
        _off = 0
        for _nm, _k, _m in spec:
            _views[_nm] = P[0:_k, _off:_off + _m]
            _off += _m
        ident = _views["ident"]
        L_E1 = _views["lhsT_E1"]
        L_rt1 = _views["lhsT_rt1"]
        B_rt1 = _views["brow_rt1"]
        L_nlg = _views["lhsT_nlog"]
        B_nlg = _views["brow_nlog"]
        L_d1 = _views["lhsT_d1"]
        B_d1 = _views["brow_d1"]
        L_d2 = _views["lhsT_d2"]
        L_d3 = _views["lhsT_d3"]
        L_g = _views["lhsT_g"]
        B_g = _views["brow_g"]
        L_c = _views["lhsT_c"]
        B_c = _views["brow_c"]
        L_a1 = _views["lhsT_a1"]
        L_a2 = _views["lhsT_a2"]
        B_a2 = _views["brow_a2"]
        h_col = _views["h_col"]
        lo5 = _views["log_obs5"]
        lR0 = _views["logR0"]
        obs11 = _views["obs11"]
        rh_p = _views["rh_p"]
        rlow_p = _views["rlow_p"]
        eh_p = _views["eh_p"]
        el_p = _views["el_p"]
        lw0_p = _views["lw0_p"]

        def act_silu(out_ap, in_ap, pool=None, shape=None, tag=None, name=None):
            if not sim_compat:
                nc.scalar.activation(out_ap, in_ap, AF.Silu)
            else:
                tmp = pool.tile(shape, F32, tag=tag, name=name or "silu_tmp")
                nc.scalar.activation(tmp[:], in_ap, AF.Sigmoid)
                nc.vector.tensor_tensor(out_ap, in_ap, tmp[:], ALU.mult)

        ones32 = sm((1, 32), "ones32")
        nc.vector.memset(ones32[:], 1.0)
        ones128 = sm((1, 128), "ones128")
        nc.vector.memset(ones128[:], 1.0)
        ones_bw = sm((1, BW), "ones_bw")
        nc.vector.memset(ones_bw[:], 1.0)
        eps_col = sm((128, 1), "eps_col")
        nc.vector.memset(eps_col[:], EPS)
        neg1_col = sm((128, 1), "neg1_col")
        nc.vector.memset(neg1_col[:], -1.0)
        two_col = sm((128, 1), "two_col")
        nc.vector.memset(two_col[:], 2.0)

        state_big = sm((128, 50 * JT), "state_big")
        stg6 = sm((128, 6 * JT), "stg6")
        stg47 = sm((128, 47 * JT), "stg47")
        hl2 = sm((128, 2 * JT), "hl2")
        w_p = sm((128, JT), "w_p")
        # pre-allocate all remaining single tiles (pool release is stack-order)
        rsr = sm((1, 1), "rsr")
        rsrc_c = sm((1, 1), "rsrc_c")
        rsrc_col = sm((128, 1), "rsrc_col")
        obs_col = sm((128, 1), "obs_col")
        e5 = sm((5, 1), "e5")
        p5 = sm((5, 1), "p5")
        L_R = sm((15, 2), "L_R")
        ah = sm((17, 1), "ah")
        al_sb = sm((1, 1), "al_sb")
        alpha_col = sm((128, 1), "alpha_col")
        asc = sm((128, 1), "asc")
        lwm = sm((128, 1), "lwm")
        lwmax_col = sm((128, 1), "lwmax_col")
        gate1 = sm((1, 1), "gate1")
        ysb = sm((50, R), "ysb")
        lwrow = sm((1, 128), "lwrow")
        lwm1 = sm((1, 1), "lwm1")

        with (
            tc.tile_pool(name="pha", bufs=1) as pha,
            tc.tile_pool(name="ck", bufs=6) as ck,
            tc.tile_pool(name="pk", bufs=24) as pk,
            tc.tile_pool(name="ppbig", bufs=2, space="PSUM") as ppbig,
            tc.tile_pool(name="ppt", bufs=2, space="PSUM") as ppt,
        ):
            # persistent phase-A buffers (pool bufs=1, unique tags).
            # All partition slices start at 0/32/64/96 (hardware AP rule).
            stack1 = pha.tile([47, N], F32, tag="stack1")   # 0-31 silu_rt1 | 32-46 adj-logits
            di = pha.tile([48, N], F32, tag="di")           # 0-31 zT | 32-47 remb
            batch = pha.tile([111, N], F32, tag="batch")    # 0-3 dp | 32-33 R | 64-95 nz | 96-110 nlog

            nc.sync.dma_start(stack1[32:47, :], d_logT[:])
            nc.sync.dma_start(di[0:32, :], d_zT[:])

            def mm_chunks(psum_t, lhsT, rhs_full, cs):
                """psum_t[:, :] = lhsT.T @ rhs_full[:, cs], in BW blocks."""
                for b in range(CH // BW):
                    bs = slice(b * BW, (b + 1) * BW)
                    gs = slice(cs.start + b * BW, cs.start + (b + 1) * BW)
                    nc.tensor.matmul(psum_t[:, bs], lhsT, rhs_full[:, gs],
                                     start=True, stop=True)

            def replicate_col(dst_col, src11, nm):
                pr = ppt.tile([128, 1], F32, tag="pt", name="rep_" + nm)
                nc.tensor.matmul(pr[:], ones128[:], src11, start=True,
                                 stop=True)
                nc.vector.tensor_copy(dst_col[:], pr[:])

            # ================= ACT set: natural_log_exp (#1) ===============
            # R_src = clip(exp(log_R[0]), .15, 2.5) broadcast to a column
            nc.scalar.activation(rsr[:], lR0, AF.Exp)
            nc.vector.tensor_scalar(rsrc_c[:], rsr[:], 0.15, 2.5, ALU.max, ALU.min)
            replicate_col(rsrc_col, rsrc_c[:], "rsrc")
            replicate_col(obs_col, obs11, "obs")
            # scales = softplus(log_obs_scale[:5]) via exp/ln (stay in set)
            nc.scalar.activation(e5[:], lo5, AF.Exp)
            nc.vector.tensor_scalar_add(p5[:], e5[:], 1.0)
            nc.vector.memset(L_R[:, 0:1], 0.0)
            nc.vector.memset(L_R[:, 1:2], 1.0)
            nc.scalar.activation(L_R[0:5, 0:1], p5[:], AF.Ln)

            for q in range(NQ):
                cs = slice(q * CH, (q + 1) * CH)
                E1_q = ck.tile([15, CH], F32, tag="ck", name="E1_q")
                nc.scalar.activation(E1_q[:], stack1[32:47, cs], AF.Exp)
                pe1 = ppbig.tile([33, CH], F32, tag="pbig", name="pe1")
                for b in range(CH // BW):
                    bs = slice(b * BW, (b + 1) * BW)
                    nc.tensor.matmul(pe1[:, bs], L_E1, E1_q[:, bs],
                                     start=True, stop=True)
                ru_q = ck.tile([16, CH], F32, tag="ck", name="ru_q")
                nc.vector.tensor_copy(ru_q[:], pe1[0:16, :])
                s1_q = ck.tile([1, CH], F32, tag="cks", bufs=2, name="s1_q")
                nc.vector.tensor_copy(s1_q[:], pe1[32:33, :])
                ps1 = ppbig.tile([32, CH], F32, tag="pbig", name="ps1")
                for b in range(CH // BW):
                    bs = slice(b * BW, (b + 1) * BW)
                    nc.tensor.matmul(ps1[:, bs], ones32[:], s1_q[:, bs],
                                     start=True, stop=True)
                rs1_q = ck.tile([32, CH], F32, tag="ck", name="rs1_q")
                nc.vector.reciprocal_approx_fast(rs1_q[:], ps1[:])
                nc.vector.tensor_tensor(di[32:48, cs], ru_q[:, :],
                                        rs1_q[0:16, :], ALU.mult)
                prt = ppbig.tile([32, CH], F32, tag="pbig", name="prt")
                for b in range(CH // BW):
                    bs = slice(b * BW, (b + 1) * BW)
                    nc.tensor.matmul(prt[:, bs], L_rt1, ru_q[:, bs],
                                     start=True, stop=False)
                    nc.tensor.matmul(prt[:, bs], B_rt1, s1_q[:, bs],
                                     start=False, stop=True)
                nc.vector.tensor_tensor(stack1[0:32, cs], prt[:], rs1_q[:],
                                        ALU.mult)

            # ================= ACT set: silu ===============================
            for q in range(NQ):
                cs = slice(q * CH, (q + 1) * CH)
                act_silu(stack1[0:32, cs], stack1[0:32, cs], ck, [32, CH], "ck")
                pd1 = ppbig.tile([64, CH], F32, tag="pbig", name="pd1")
                for b in range(CH // BW):
                    bs = slice(b * BW, (b + 1) * BW)
                    gs = slice(cs.start + b * BW, cs.start + (b + 1) * BW)
                    nc.tensor.matmul(pd1[:, bs], L_d1, di[:, gs],
                                     start=True, stop=False)
                    nc.tensor.matmul(pd1[:, bs], B_d1, ones_bw[:],
                                     start=False, stop=True)
                a1_q = ck.tile([65, CH], F32, tag="ck", name="a1_q")
                nc.vector.memset(a1_q[64:65, :], 1.0)
                act_silu(a1_q[0:64, :], pd1[:], ck, [64, CH], "ck")
                pd2 = ppbig.tile([32, CH], F32, tag="pbig", name="pd2")
                for b in range(CH // BW):
                    bs = slice(b * BW, (b + 1) * BW)
                    nc.tensor.matmul(pd2[:, bs], L_d2, a1_q[:, bs],
                                     start=True, stop=True)
                a2_q = ck.tile([33, CH], F32, tag="ck", name="a2_q")
                nc.vector.memset(a2_q[32:33, :], 1.0)
                act_silu(a2_q[0:32, :], pd2[:], ck, [32, CH], "ck")
                pd3 = ppt.tile([4, CH], F32, tag="pt", name="pd3")
                for b in range(CH // BW):
                    bs = slice(b * BW, (b + 1) * BW)
                    nc.tensor.matmul(pd3[:, bs], L_d3, a2_q[:, bs],
                                     start=True, stop=True)
                nc.vector.tensor_copy(batch[0:4, cs], pd3[:])
            # alpha (scalar path, stays in silu set)
            pa1 = ppt.tile([16, 1], F32, tag="pt", name="pa1")
            nc.tensor.matmul(pa1[:], L_a1, h_col, start=True, stop=True)
            act_silu(ah[0:16, :], pa1[:], pk, [16, 1], "pksmall")
            pal = ppt.tile([1, 1], F32, tag="pt", name="pal")
            nc.tensor.matmul(pal[:], L_a2, ah[0:16, :],
                             start=True, stop=False)
            nc.tensor.matmul(pal[:], B_a2, ones32[0:1, 0:1],
                             start=False, stop=True)
            nc.vector.tensor_copy(al_sb[:], pal[:])
            replicate_col(alpha_col, al_sb[:], "alpha")
            nc.vector.tensor_scalar_mul(asc[:], alpha_col[:], INV_SQRT2)

            # ================= ACT set: natural_log_exp (#2) ===============
            for q in range(NQ):
                cs = slice(q * CH, (q + 1) * CH)
                pnl = ppbig.tile([15, CH], F32, tag="pbig", name="pnl")
                for b in range(CH // BW):
                    bs = slice(b * BW, (b + 1) * BW)
                    gs = slice(cs.start + b * BW, cs.start + (b + 1) * BW)
                    nc.tensor.matmul(pnl[:, bs], L_nlg, stack1[:, gs],
                                     start=True, stop=False)
                    nc.tensor.matmul(pnl[:, bs], B_nlg, ones_bw[:],
                                     start=False, stop=True)
                E2_q = ck.tile([15, CH], F32, tag="ck", name="E2_q")
                nc.scalar.activation(E2_q[:], pnl[:], AF.Exp)
                nc.vector.tensor_copy(batch[96:111, cs], pnl[:])
                pR = ppt.tile([2, CH], F32, tag="pt", name="pR")
                for b in range(CH // BW):
                    bs = slice(b * BW, (b + 1) * BW)
                    nc.tensor.matmul(pR[:, bs], L_R[:], E2_q[:, bs],
                                     start=True, stop=True)
                nc.vector.tensor_copy(batch[32:34, cs], pR[:])

            # ---- transpose dp/R rows -> stg6 (packed, partition-minor) ----
            for m in range(JT):
                mb = slice(m * 128, (m + 1) * 128)
                pta = ppt.tile([128, 34], F32, tag="pt", name="pta")
                nc.tensor.transpose(pta[:], batch[0:34, mb], ident[0:34, 0:34])
                nc.vector.tensor_copy(stg6[:, m * 6:m * 6 + 4], pta[:, 0:4])
                nc.vector.tensor_copy(stg6[:, m * 6 + 4:m * 6 + 6],
                                      pta[:, 32:34])

            # ---- packed scalar chain (all [128, JT]) ----------------------
            dp0v = stg6[:, 0:6 * JT:6]
            dp1v = stg6[:, 1:6 * JT:6]
            dp2v = stg6[:, 2:6 * JT:6]
            dp3v = stg6[:, 3:6 * JT:6]
            Rnv = stg6[:, 4:6 * JT:6]
            Rdv = stg6[:, 5:6 * JT:6]
            nhv = hl2[:, 0:2 * JT:2]
            nlv = hl2[:, 1:2 * JT:2]

            def pkt(name):
                return pk.tile([128, JT], F32, tag="pk", name=name)

            # sig_h/sig_l = softplus(dp2/3)+0.01 via exp/ln (nat set)
            for dpv, epsv, rv, outv in ((dp2v, eh_p, rh_p, nhv),
                                        (dp3v, el_p, rlow_p, nlv)):
                ex = pkt("ex")
                nc.scalar.activation(ex[:], dpv, AF.Exp)
                ex2 = pkt("ex2")
                nc.vector.tensor_scalar_add(ex2[:], ex[:], 1.0)
                sp = pkt("sp")
                nc.scalar.activation(sp[:], ex2[:], AF.Ln)
                m1 = pkt("m1")
                nc.vector.scalar_tensor_tensor(m1[:], sp[:], 0.01, epsv[:],
                                               ALU.add, ALU.mult)
                s1 = pkt("s1")
                nc.vector.tensor_tensor(s1[:], m1[:], rv[:], ALU.add)
                s2 = pkt("s2")
                nc.vector.tensor_tensor(s2[:], s1[:],
                                        dp0v if outv is nhv else dp1v, ALU.add)
                nc.vector.tensor_scalar_max(outv, s2[:], 0.0)

            # R = clip(R_src * Rn/Rd, .15, 4)
            rdr = pkt("rdr")
            nc.vector.reciprocal(rdr[:], Rdv)
            rr1 = pkt("rr1")
            nc.vector.tensor_tensor(rr1[:], rdr[:], Rnv, ALU.mult)
            Rv0 = pkt("Rv0")
            nc.vector.tensor_scalar(Rv0[:], rr1[:], rsrc_col[:, 0:1], None,
                                    ALU.mult)
            Rv = pkt("Rv")
            nc.vector.tensor_scalar(Rv[:], Rv0[:], 0.15, 4.0, ALU.max, ALU.min)
            rcpR = pkt("rcpR")
            nc.vector.reciprocal(rcpR[:], Rv[:])
            # zz = (obs - nh)/R ; x = alpha*zz/sqrt(2)
            zzt = pkt("zzt")
            nc.vector.tensor_scalar(zzt[:], nhv, obs_col[:, 0:1], -1.0,
                                    ALU.subtract, ALU.mult)
            zz = pkt("zz")
            nc.vector.tensor_tensor(zz[:], zzt[:], rcpR[:], ALU.mult)
            xw = pkt("xw")
            nc.vector.tensor_scalar(xw[:], zz[:], asc[:, 0:1], None, ALU.mult)

            # ================= ACT set: sigmoid ============================
            for q in range(NQ):
                cs = slice(q * CH, (q + 1) * CH)
                pg = ppbig.tile([32, CH], F32, tag="pbig", name="pg")
                for b in range(CH // BW):
                    bs = slice(b * BW, (b + 1) * BW)
                    gs = slice(cs.start + b * BW, cs.start + (b + 1) * BW)
                    nc.tensor.matmul(pg[:, bs], L_g, di[:, gs],
                                     start=True, stop=False)
                    nc.tensor.matmul(pg[:, bs], B_g, ones_bw[:],
                                     start=False, stop=True)
                gate_q = ck.tile([32, CH], F32, tag="ck", name="gate_q")
                nc.scalar.activation(gate_q[:], pg[:], AF.Sigmoid)
                pc = ppbig.tile([32, CH], F32, tag="pbig", name="pc")
                for b in range(CH // BW):
                    bs = slice(b * BW, (b + 1) * BW)
                    gs = slice(cs.start + b * BW, cs.start + (b + 1) * BW)
                    nc.tensor.matmul(pc[:, bs], L_c, di[:, gs],
                                     start=True, stop=False)
                    nc.tensor.matmul(pc[:, bs], B_c, ones_bw[:],
                                     start=False, stop=True)
                th_q = ck.tile([32, CH], F32, tag="ck", name="th_q")
                nc.scalar.activation(th_q[:], pc[:], AF.Tanh)
                dq = ck.tile([32, CH], F32, tag="ck", name="dq")
                nc.vector.tensor_tensor(dq[:], di[0:32, cs], th_q[:],
                                        ALU.subtract)
                pq = ck.tile([32, CH], F32, tag="ck", name="pq")
                nc.vector.tensor_tensor(pq[:], gate_q[:], dq[:], ALU.mult)
                nc.vector.tensor_tensor(batch[64:96, cs], th_q[:], pq[:],
                                        ALU.add)
            erf_t = pkt("erf_t")
            nc.scalar.activation(erf_t[:], xw[:], ERF)
            nd = pkt("nd")
            nc.vector.tensor_scalar(nd[:], erf_t[:], 0.5, 0.5, ALU.mult,
                                    ALU.add)

            # ---- transpose nz/nlog rows -> stg47 --------------------------
            for m in range(JT):
                mb = slice(m * 128, (m + 1) * 128)
                ptb = ppt.tile([128, 47], F32, tag="pt", name="ptb")
                nc.tensor.transpose(ptb[:], batch[64:111, mb],
                                    ident[64:111, 64:111])
                nc.vector.tensor_copy(stg47[:, m * 47:(m + 1) * 47], ptb[:])

            # ================= ACT set: natural_log_exp (#3) ===============
            lc = pkt("lc")
            nc.scalar.activation(lc[:], nd[:], AF.Ln)
            lnR = pkt("lnR")
            nc.scalar.activation(lnR[:], Rv[:], AF.Ln)
            zz2 = pkt("zz2")
            nc.vector.tensor_tensor(zz2[:], zz[:], zz[:], ALU.mult)
            l1 = pkt("l1")
            nc.vector.scalar_tensor_tensor(l1[:], zz2[:], -0.5, lc[:],
                                           ALU.mult, ALU.add)
            l2 = pkt("l2")
            nc.vector.scalar_tensor_tensor(l2[:], lnR[:], -1.0, l1[:],
                                           ALU.mult, ALU.add)
            lw = pkt("lw")
            nc.vector.scalar_tensor_tensor(lw[:], lw0_p, C_LL, l2[:],
                                           ALU.add, ALU.add)
            nc.vector.tensor_reduce(lwm[:], lw[:], AX.X, ALU.max)
            ptl = ppt.tile([1, 128], F32, tag="pt", name="ptl")
            nc.tensor.transpose(ptl[:], lwm[:], ident)
            nc.vector.tensor_copy(lwrow[:], ptl[:])
            nc.vector.tensor_reduce(lwm1[:], lwrow[:], AX.X, ALU.max)
            replicate_col(lwmax_col, lwm1[:], "lwmax")
            dsh = pkt("dsh")
            nc.vector.tensor_scalar(dsh[:], lw[:], lwmax_col[:, 0:1], LWCLAMP,
                                    ALU.subtract, ALU.max)
            nc.scalar.activation(w_p[:], dsh[:], AF.Exp, scale=two_col[:])

            # ---- state assembly: state_w tiles [128, 50] per j-tile -------
            for m in range(JT):
                st = state_big[:, m * 50:(m + 1) * 50]
                wc = w_p[:, m:m + 1]
                nc.vector.tensor_scalar(st[:, 0:2], hl2[:, 2 * m:2 * m + 2],
                                        wc, None, ALU.mult)
                nc.vector.tensor_scalar(st[:, 2:49],
                                        stg47[:, m * 47:(m + 1) * 47],
                                        wc, None, ALU.mult)
                nc.vector.tensor_copy(st[:, 49:50], wc)

            # ordering gate: force big-loop Ln after all phase-A ACT work
            nc.vector.tensor_scalar(gate1[:], w_p[0:1, 0:1], 0.0, 1.0e30,
                                    ALU.mult, ALU.add)

        # ================= big loop ========================================
        with (
            tc.tile_pool(name="blu", bufs=2) as blu,
            tc.tile_pool(name="blt", bufs=2) as blt,
            tc.tile_pool(name="pyp", bufs=1, space="PSUM") as pyp,
            tc.tile_pool(name="pout", bufs=2, space="PSUM") as pout,
        ):
            py = pyp.tile([50, R], F32, tag="py")
            uT_r = d_uT.rearrange("(s k p) c -> s p k c", p=128, k=G)
            for s in range(SUP):
                u_sup = blu.tile([128, G * R], F32, tag="u", name="u_sup")
                nc.sync.dma_start(
                    u_sup.rearrange("p (k c) -> p k c", k=G), uT_r[s])
                nc.vector.tensor_scalar(u_sup[0:1, 0:1], u_sup[0:1, 0:1],
                                        gate1[0:1, 0:1], None, ALU.min)
                t_sup = blt.tile([128, G * R], F32, tag="t", name="t_sup")
                nc.scalar.activation(t_sup[:], u_sup[:], AF.Ln, bias=eps_col[:])
                nc.scalar.activation(u_sup[:], t_sup[:], AF.Square,
                                     bias=eps_col[:], scale=neg1_col[:])
                nc.vector.reciprocal_approx_fast(t_sup[:], u_sup[:])
                for k in range(G):
                    jt = s * G + k
                    lhsT = state_big[:, jt * 50:(jt + 1) * 50]
                    for b in range(NB):
                        rs = slice(k * R + b * MB, k * R + (b + 1) * MB)
                        ps = slice(b * MB, (b + 1) * MB)
                        nc.tensor.matmul(py[:, ps], lhsT, t_sup[:, rs],
                                         start=(jt == 0), stop=(jt == JT - 1))

            # ---- output: transpose back, divide by denominator ------------
            nc.vector.tensor_copy(ysb[:], py[:])
            with tc.tile_pool(name="outp", bufs=2) as outp:
                for ob in range(OB):
                    obs_ = slice(ob * OW, (ob + 1) * OW)
                    po = pout.tile([OW, 50], F32, tag="po", name="po")
                    nc.tensor.transpose(po[:], ysb[:, obs_], ident[0:50, 0:50])
                    osb = outp.tile([OW, 50], F32, tag="osb", name="osb")
                    nc.vector.tensor_copy(osb[:], po[:])
                    rden = outp.tile([OW, 1], F32, tag="rden", name="rden")
                    nc.vector.reciprocal(rden[:], osb[:, 49:50])
                    yt = outp.tile([OW, 49], F32, tag="yt", name="yt")
                    nc.vector.tensor_scalar(yt[:], osb[:, 0:49], rden[:, 0:1],
                                            None, ALU.mult)
                    nc.sync.dma_start(d_y[obs_, :], yt[:])

        # release the single-tile pools in reverse creation order
        for free in reversed(_keep):
            free()

    nc.compile()
    return nc


# ---------------------------------------------------------------------------
# host-side preparation
# ---------------------------------------------------------------------------

def _f32(x):
    return np.ascontiguousarray(np.asarray(x, dtype=np.float32))


def prep_inputs(inputs, n_cores):
    """Returns (common dict, list of per-core dicts)."""
    g = {k: _f32(v) for k, v in inputs.items()}
    N = g["z"].shape[0]
    JT = N // 128
    R = N // n_cores
    h = g["h_t"]

    def packed(a):
        return np.ascontiguousarray(a.reshape(JT, 128).T)

    W_rt1, W_d1, W_g, W_c = g["W_rt1"], g["W_d1"], g["W_g"], g["W_c"]
    b_rt1 = g["b_rt1"] + W_rt1[:, :64] @ h
    b_d1 = g["b_d1"] + W_d1[:, :64] @ h
    b_g = g["b_g"] + W_g[:, :64] @ h
    b_c = g["b_c"] + W_c[:, :64] @ h

    # out rows: 0-15 = remb_un, 32 = S1 (sum of all 15 exps)
    lhsT_E1 = np.zeros((15, 33), np.float32)
    lhsT_E1[:K_ACT, 0:16] = g["embed"][:K_ACT]
    lhsT_E1[:, 32] = 1.0

    lhsT_rt1 = _f32(W_rt1[:, 64:80].T)
    brow_rt1 = _f32(b_rt1[None, :])

    brow_nlog = np.zeros((1, 15), np.float32)
    brow_nlog[0, :K_ACT] = 0.3 * g["b_rt2"][:K_ACT]

    lhsT_nlog = np.zeros((47, 15), np.float32)
    lhsT_nlog[0:32, :K_ACT] = 0.3 * g["W_rt2"].T[:, :K_ACT]
    for c in range(15):
        lhsT_nlog[32 + c, c] = 0.7 if c < K_ACT else 1.0

    # di rows: 0-31 z, 32-47 remb
    lhsT_d1 = np.concatenate([W_d1[:, 80:112].T, W_d1[:, 64:80].T], 0)
    brow_d1 = _f32(b_d1[None, :])
    lhsT_d2 = np.concatenate([g["W_d2"].T, g["b_d2"][None, :]], 0)
    lhsT_d3 = np.concatenate([g["W_d3"].T, g["b_d3"][None, :]], 0)
    lhsT_g = np.concatenate([W_g[:, 80:112].T, W_g[:, 64:80].T], 0)
    brow_g = _f32(b_g[None, :])
    lhsT_c = np.concatenate([W_c[:, 80:112].T, W_c[:, 64:80].T], 0)
    brow_c = _f32(b_c[None, :])
    lhsT_a1 = np.concatenate([g["W_a1"].T, g["b_a1"][None, :]], 0)
    lhsT_a2 = _f32(g["W_a2"].T)
    brow_a2 = _f32(g["b_a2"][None, :])
    h_colv = np.concatenate([h, np.ones(1, np.float32)])[:, None]

    pieces = {
        "ident": np.eye(128, dtype=np.float32),
        "lhsT_E1": _f32(lhsT_E1), "lhsT_rt1": lhsT_rt1, "brow_rt1": brow_rt1,
        "lhsT_nlog": _f32(lhsT_nlog), "brow_nlog": brow_nlog,
        "lhsT_d1": _f32(lhsT_d1), "brow_d1": brow_d1,
        "lhsT_d2": _f32(lhsT_d2), "lhsT_d3": _f32(lhsT_d3),
        "lhsT_g": _f32(lhsT_g), "brow_g": brow_g,
        "lhsT_c": _f32(lhsT_c), "brow_c": brow_c,
        "lhsT_a1": _f32(lhsT_a1), "lhsT_a2": lhsT_a2, "brow_a2": brow_a2,
        "h_col": _f32(h_colv),
        "log_obs5": _f32(g["log_obs_scale"][:K_ACT][:, None]),
        "logR0": _f32(g["log_R"][0].reshape(1, 1)),
        "obs11": _f32(np.asarray(g["obs_remaining"]).reshape(1, 1)),
        "rh_p": packed(g["remaining_high"]),
        "rlow_p": packed(g["remaining_low"]),
        "eh_p": packed(g["eps_high"]),
        "el_p": packed(g["eps_low"]),
        "lw0_p": packed(g["log_weights"]),
    }
    spec = _param_spec(JT)
    CP = sum(m for _, _, m in spec)
    params = np.zeros((128, CP), np.float32)
    off = 0
    for nm, k, m in spec:
        arr = pieces[nm]
        assert arr.shape == (k, m), (nm, arr.shape, (k, m))
        params[0:k, off:off + m] = arr
        off += m

    common = dict(
        zT=np.ascontiguousarray(g["z"].T),
        logitsT=np.ascontiguousarray(g["regime_logits"].T),
        params=params,
    )

    u = g["u_gumbel"]
    in_maps = []
    for c in range(n_cores):
        m = dict(common)
        m["uT"] = np.ascontiguousarray(u[c * R:(c + 1) * R, :].T)
        in_maps.append(m)
    return in_maps


_PROG_CACHE = {}
TRACE = False           # set True (e.g. from test.py) to profile on HW
LAST_EXEC_NS = None


def kernel(**inputs):
    global LAST_EXEC_NS
    n_cores = 8
    N = int(np.asarray(inputs["z"]).shape[0])
    R = N // n_cores
    key = (N, R)
    if key not in _PROG_CACHE:
        _PROG_CACHE[key] = build_program(N, R)
    nc = _PROG_CACHE[key]
    in_maps = prep_inputs(inputs, n_cores)
    res = run_bass_kernel_spmd(nc, in_maps, list(range(n_cores)),
                               trace=TRACE)
    LAST_EXEC_NS = res.exec_time_ns
    outs = [res.results[c]["y"] for c in range(n_cores)]
    return np.concatenate(outs, axis=0).astype(np.float32)



# revision 7
# speedup vs baseline: 3.6010x; 3.6010x over previous
"""Trainium2 Bass kernel for nn_DifferentiableParticleFilter (N=8192, 8 cores).

Sharding: the (N,N) soft-resample matrix is sharded by output rows (1024 per
core); the per-particle network + state (N,49) is computed replicated on each
core.  Host pre-transposes each u_gumbel shard so the contraction axis lands
on SBUF partitions.

Big-tensor pipeline per supertile (tau = 0.5 -> exp(g/tau) = 1/ln(u+eps)^2):
    DMA u (fp32) -> scalar Ln in place -> fused custom DVE op
    t = (1/L)^2 (bitnot exponent-flip seed + one tuned NR step + square,
    bf16 out) -> bf16 PE matmul against the weight-folded state.
Phase A (per-particle nets) runs in a 4-way particle-stacked layout
([4*d rows, 2048 cols]) with bf16 matmuls; biases fold into activation bias
columns; particle weights fold into the state via one broadcast TT; the
sigmoid gate is computed as (1+tanh(x/2))/2 so every phase-A transcendental
sits in the silu/tanh table set (4 ACT set switches total).
"""

import numpy as np

import concourse.bass as bass
import concourse.tile as tile
from concourse import bacc, mybir
from concourse.bass_utils import run_bass_kernel_spmd

F32 = mybir.dt.float32
BF16 = mybir.dt.bfloat16
AF = mybir.ActivationFunctionType
ALU = mybir.AluOpType
AX = mybir.AxisListType

K_ACT = 5
EPS = 1.0e-10
LWCLAMP = -30.0
C_LL = float(np.log(2.0) - 0.5 * np.log(2.0 * np.pi))
INV_SQRT2 = float(1.0 / np.sqrt(2.0))

# fused DVE op: out = s0 * ( not_x*(A - B*(x*not_x)) )^2  ~=  s0/x^2
# A,B minimax-tuned for one NR step over u = x*bitcast(~x) in [-4.5,-4].
RSQ_A = -0.47141455934487236
RSQ_B = 0.05546133703759453

B4 = 4                  # particle-stacking factor for phase A
_RSQ_OP = None


def _register_recip_sq():
    """Register the fused (1/x)^2 custom DVE op (idempotent)."""
    global _RSQ_OP
    if _RSQ_OP is not None:
        return _RSQ_OP
    import concourse.dve_ops as dve_ops
    from concourse.dve_ops import DveOp
    from concourse.dve_spec import (AluOp, Bin, C0, C1, C2, Spec, Src0,
                                    lower, _has_src1)
    from concourse.dve_uop import DveOpSpec

    name = "RECIP_SQ_W_ANT"
    for op in dve_ops.OPS:
        if op.name == name:
            _RSQ_OP = op
            return op

    _not = Bin(AluOp.BITWISE_NOT, Src0, Src0)
    _y1 = _not * (C2 - C1 * (Src0 * _not))
    _body = (_y1 * _y1) * C0

    def _ref(in0, in1, c0, c1, c2):
        not_x = (~in0.view(np.int32)).view(np.float32)
        y1 = not_x * (c2 - c1 * (in0 * not_x))
        return (y1 * y1) * c0

    spec = Spec(body=_body, reference=_ref)
    opcode = dve_ops._CUSTOM_DVE_ROW_BASE + len(dve_ops.OPS)
    shas = {}
    for ver in ("v3", "v4"):
        try:
            uops = lower(spec, ver=ver)
            shas[ver] = DveOpSpec(
                name=name, uops=uops, opcode=opcode, rd1_en=_has_src1(spec)
            ).sha(ver)
        except Exception:
            pass
    op = DveOp(name, spec, subdim=False, uops_sha=shas)
    dve_ops.OPS.append(op)
    dve_ops.CUSTOM_DVE_SPECS[name] = spec
    dve_ops._SUB_OPCODE_FOR_NAME[name] = opcode
    _RSQ_OP = op
    return op


# fp32 parameter blob (one DMA): (name, n_partitions, n_cols)
def _param_spec_f32():
    return [
        ("ident", 128, 128),
        ("lhsT_a1", 65, 16), ("lhsT_a2", 16, 1), ("brow_a2", 1, 1),
        ("h_col", 65, 1), ("lhsT_rep5", 5, 128),
        ("log_obs5", 5, 1), ("logR0", 1, 1), ("obs11", 1, 1),
        ("b_x1", 128, 1), ("b_d1h0", 128, 1), ("b_d1h1", 128, 1),
        ("b_d2", 128, 1), ("b_dR", 40, 1), ("b_nlg", 128, 1),
        ("b_g", 128, 1), ("b_c", 128, 1),
        ("rh_p", 128, 64), ("rlow_p", 128, 64), ("eh_p", 128, 64),
        ("el_p", 128, 64), ("lw0_p", 128, 64),
    ]


# bf16 parameter blob: block-diagonal lhsTs for the 4-way stacked layout
def _param_spec_bf16():
    return [
        ("identb", 128, 128),
        ("lhsT_E1r", 60, 128), ("lhsT_E1s", 60, 128),
        ("lhsT_x1", 128, 128),
        ("lhsT_nlgx", 128, 128), ("lhsT_nlgl", 60, 128),
        ("lhsT_d1r0", 128, 128), ("lhsT_d1z0", 128, 128),
        ("lhsT_d1r1", 128, 128), ("lhsT_d1z1", 128, 128),
        ("lhsT_d2a0", 128, 128), ("lhsT_d2a1", 128, 128),
        ("lhsT_d3", 128, 16),
        ("lhsT_gr", 128, 128), ("lhsT_gz", 128, 128),
        ("lhsT_cr", 128, 128), ("lhsT_cz", 128, 128),
    ]


# ---------------------------------------------------------------------------
# device program (SPMD - one program, per-core inputs differ)
# ---------------------------------------------------------------------------

def build_program(n_particles, rows_per_core):
    N = int(n_particles)
    R = int(rows_per_core)
    JT = N // 128                 # 64 j-tiles (contraction tiles of 128)
    CB = N // B4                  # stacked cols per block (2048)
    NCH = CB // 1024              # phase-A 1024-col psum chunks (2)
    G = 4                         # j-tiles per supertile
    SUP = JT // G                 # 16 supertiles
    TW = G * R                    # supertile width (4096)
    NTC = CB // 128               # transpose chunks (16)
    OW = min(128, R)
    OB = R // OW

    rsq = _register_recip_sq()

    nc = bacc.Bacc("TRN2", target_bir_lowering=False, debug=False)

    def par(name, shape, dt=F32, out=False):
        return nc.declare_dram_parameter(name, list(shape), dt, isOutput=out)

    specf = _param_spec_f32()
    CPf = sum(m for _, _, m in specf)
    specb = _param_spec_bf16()
    CPb = sum(m for _, _, m in specb)
    d_uT = par("uT", (N, R))
    d_z4 = par("z4", (128, CB), BF16)
    d_lg4 = par("lg4", (60, CB), BF16)
    d_pf = par("pf", (128, CPf))
    d_pb = par("pb", (128, CPb), BF16)
    d_y = par("y", (R, 49), out=True)

    with tile.TileContext(nc) as tc:
        _keep = []

        def sm(shape, name, dt=F32):
            t, free = tc.tile(list(shape), dt, name=name)
            _keep.append(free)
            return t

        # ---- persistent tiles -------------------------------------------
        Pf = sm((128, CPf), "Pf")
        nc.sync.dma_start(Pf[:], d_pf[:])
        Pb = sm((128, CPb), "Pb", BF16)
        nc.sync.dma_start(Pb[:], d_pb[:])

        def views(P, spec):
            v, off = {}, 0
            for nm, k, m in spec:
                v[nm] = P[0:k, off:off + m]
                off += m
            return v

        Vf = views(Pf, specf)
        Vb = views(Pb, specb)
        ident = Vf["ident"]
        identb = Vb["identb"]

        state = sm((128, 50 * JT), "state", BF16)
        stg6 = sm((128, 6 * JT), "stg6")
        hl2 = sm((128, 2 * JT), "hl2")
        w_p = sm((128, JT), "w_p")
        eps_col = sm((128, 1), "eps_col")
        nc.vector.memset(eps_col[:], EPS)
        one_col = sm((128, 1), "one_col")
        nc.vector.memset(one_col[:], 1.0)
        ones128 = sm((1, 128), "ones128")
        nc.vector.memset(ones128[:], 1.0)
        L_R4 = sm((128, 8), "L_R4", BF16)
        nc.vector.memset(L_R4[:], 0.0)
        rsr = sm((1, 1), "rsr")
        rsrc_col = sm((128, 1), "rsrc_col")
        obs_col = sm((128, 1), "obs_col")
        e5 = sm((5, 1), "e5")
        scl_col = sm((128, 1), "scl_col")
        ah = sm((16, 1), "ah")
        al_sb = sm((1, 1), "al_sb")
        alpha_col = sm((128, 1), "alpha_col")
        asc = sm((128, 1), "asc")
        lwm = sm((128, 1), "lwm")
        lwmax_col = sm((128, 1), "lwmax_col")
        lwrow = sm((1, 128), "lwrow")
        lwm1 = sm((1, 1), "lwm1")
        ysb = sm((50, R), "ysb")

        statemv = state[:, :].rearrange("p (m f) -> p m f", m=JT)
        statebv = state[:, :].rearrange("p (b x) -> p b x", b=B4)
        stg6bv = stg6[:, :].rearrange("p (b x) -> p b x", b=B4)

        # ---- streaming pools (outlive phase A) --------------------------
        with (
            tc.tile_pool(name="ust", bufs=4) as ust,
            tc.tile_pool(name="tst", bufs=6) as tst,
        ):
            uT_r = d_uT.rearrange("(s k p) c -> s p k c", p=128, k=G)
            u_tiles = []
            t_tiles = []
            for s in range(SUP):
                ut = ust.tile([128, TW], F32, tag="u", name=f"u{s}")
                nc.sync.dma_start(ut.rearrange("p (k c) -> p k c", k=G),
                                  uT_r[s])
                u_tiles.append(ut)
                t_tiles.append(
                    tst.tile([128, TW], BF16, tag="t", name=f"t{s}"))

            def ln_sup(s):
                nc.scalar.activation(u_tiles[s][:], u_tiles[s][:], AF.Ln,
                                     bias=eps_col[:])

            def rsq_sup(s):
                nc.vector._custom_dve(rsq, out=t_tiles[s][:],
                                      in0=u_tiles[s][:],
                                      s0=1.0, s1=RSQ_B, imm2=RSQ_A)

            # =================== phase A =================================
            with (
                tc.tile_pool(name="pha", bufs=1) as pha,
                tc.tile_pool(name="pr2", bufs=1) as pr2,
            ):
                from contextlib import ExitStack
                _psk = ExitStack()
                ppA = _psk.enter_context(
                    tc.tile_pool(name="ppA", bufs=2, space="PSUM"))
                ppB = _psk.enter_context(
                    tc.tile_pool(name="ppB", bufs=1, space="PSUM"))
                ppt = _psk.enter_context(
                    tc.tile_pool(name="ppt", bufs=1, space="PSUM"))
                lg4 = pha.tile([60, CB], BF16, tag="lg4")
                nc.sync.dma_start(lg4[:], d_lg4[:])
                z4 = pha.tile([128, CB], BF16, tag="z4")
                nc.sync.dma_start(z4[:], d_z4[:])

                # --- scalar NL group 1 -----------------------------------
                ln_sup(0)
                ln_sup(1)
                Esb = pha.tile([60, CB], BF16, tag="Esb")
                nc.scalar.activation(Esb[:], lg4[:], AF.Exp)
                nc.scalar.activation(e5[:], Vf["log_obs5"], AF.Exp)
                nc.scalar.activation(rsr[:], Vf["logR0"], AF.Exp)

                def mms(psum_t, pairs, cs, rows=slice(0, 128)):
                    """psum_t[rows,:1024] += sum_i lhsT_i.T @ rhs_i[:, cs],
                    as 2x512-col matmuls (one PSUM bank each)."""
                    for b5 in range(2):
                        bs = slice(b5 * 512, (b5 + 1) * 512)
                        gs = slice(cs.start + b5 * 512,
                                   cs.start + (b5 + 1) * 512)
                        for i, (lt, rh) in enumerate(pairs):
                            nc.tensor.matmul(psum_t[rows, bs], lt,
                                             rh[:, gs],
                                             start=(i == 0),
                                             stop=(i == len(pairs) - 1))

                def mlp_layer(out_sb, pairs, af, bias_ap, nm):
                    for ch in range(NCH):
                        cs = slice(ch * 1024, (ch + 1) * 1024)
                        ps = ppA.tile([128, 1024], F32, tag="pA",
                                      name=f"{nm}{ch}")
                        mms(ps, pairs, cs)
                        if bias_ap is None:
                            nc.scalar.activation(out_sb[:, cs], ps[:], af)
                        else:
                            nc.scalar.activation(out_sb[:, cs], ps[:], af,
                                                 bias=bias_ap)

                # --- E1: remb = (E @ embed5) / S1 ------------------------
                remb = pha.tile([128, CB], BF16, tag="remb")
                for ch in range(NCH):
                    cs = slice(ch * 1024, (ch + 1) * 1024)
                    p_r = ppA.tile([128, 1024], F32, tag="pA",
                                   name=f"p_remb{ch}")
                    mms(p_r, [(Vb["lhsT_E1r"], Esb)], cs)
                    p_s = ppA.tile([128, 1024], F32, tag="pA",
                                   name=f"p_s1{ch}")
                    mms(p_s, [(Vb["lhsT_E1s"], Esb)], cs)
                    r2 = pr2.tile([128, 1024], F32, tag="r2",
                                  name=f"r2{ch}")
                    nc.vector.reciprocal_approx_fast(r2[:], p_s[:])
                    nc.vector.tensor_tensor(remb[:, cs], p_r[:], r2[:],
                                            ALU.mult)

                # custom op for supertiles 0,1 (frees their u buffers)
                rsq_sup(0)
                rsq_sup(1)

                # --- scalar SILU/TANH group ------------------------------
                x1 = pha.tile([128, CB], BF16, tag="x1")
                mlp_layer(x1, [(Vb["lhsT_x1"], remb)], AF.Silu,
                          Vf["b_x1"], "p_x1")
                a1h0 = pha.tile([128, CB], BF16, tag="a1h0")
                mlp_layer(a1h0, [(Vb["lhsT_d1r0"], remb),
                                 (Vb["lhsT_d1z0"], z4)], AF.Silu,
                          Vf["b_d1h0"], "p_d1a")
                a1h1 = pha.tile([128, CB], BF16, tag="a1h1")
                mlp_layer(a1h1, [(Vb["lhsT_d1r1"], remb),
                                 (Vb["lhsT_d1z1"], z4)], AF.Silu,
                          Vf["b_d1h1"], "p_d1b")
                a2 = pha.tile([128, CB], BF16, tag="a2")
                mlp_layer(a2, [(Vb["lhsT_d2a0"], a1h0),
                               (Vb["lhsT_d2a1"], a1h1)], AF.Silu,
                          Vf["b_d2"], "p_d2")
                th = pha.tile([128, CB], BF16, tag="th")
                mlp_layer(th, [(Vb["lhsT_gr"], remb),
                               (Vb["lhsT_gz"], z4)], AF.Tanh,
                          Vf["b_g"], "p_g")
                cand = pha.tile([128, CB], BF16, tag="cand")
                mlp_layer(cand, [(Vb["lhsT_cr"], remb),
                                 (Vb["lhsT_cz"], z4)], AF.Tanh,
                          Vf["b_c"], "p_c")

                # --- alpha (scalar path, silu group) ---------------------
                pa1 = ppt.tile([16, 1], F32, tag="pt", name="pa1")
                nc.tensor.matmul(pa1[:], Vf["lhsT_a1"], Vf["h_col"],
                                 start=True, stop=True)
                nc.scalar.activation(ah[:], pa1[:], AF.Silu)
                pal = ppt.tile([1, 1], F32, tag="pt", name="pal")
                nc.tensor.matmul(pal[:], Vf["lhsT_a2"], ah[:],
                                 start=True, stop=False)
                nc.tensor.matmul(pal[:], Vf["brow_a2"],
                                 one_col[0:1, 0:1], start=False, stop=True)
                nc.vector.tensor_copy(al_sb[:], pal[:])

                def replicate_col(dst_col, src11, nm):
                    pr = ppt.tile([128, 1], F32, tag="pt", name="rep_" + nm)
                    nc.tensor.matmul(pr[:], ones128[:], src11, start=True,
                                     stop=True)
                    nc.vector.tensor_copy(dst_col[:], pr[:])

                replicate_col(alpha_col, al_sb[:], "alpha")
                nc.vector.tensor_scalar_mul(asc[:], alpha_col[:], INV_SQRT2)
                replicate_col(obs_col, Vf["obs11"], "obs")
                nc.vector.tensor_scalar(rsr[:], rsr[:], 0.15, 2.5,
                                        ALU.max, ALU.min)
                replicate_col(rsrc_col, rsr[:], "rsrc")

                # --- nz on gpsimd: nz = cand + 0.5*(1+th)*(z-cand) -------
                nzt = pha.tile([128, CB], BF16, tag="nzt")
                nc.gpsimd.tensor_tensor(nzt[:], z4[:], cand[:], ALU.subtract)
                nc.vector.scalar_tensor_tensor(nzt[:], th[:], 1.0, nzt[:],
                                               ALU.add, ALU.mult)
                nz = pha.tile([128, CB], BF16, tag="nz")
                nc.vector.scalar_tensor_tensor(nz[:], nzt[:], 0.5, cand[:],
                                               ALU.mult, ALU.add)

                # --- nlg -> E2 / new_logits (scalar NL group 2) ----------
                E2 = pha.tile([128, CB], BF16, tag="E2")
                nlogsb = pha.tile([128, CB], BF16, tag="nlogsb")
                for ch in range(NCH):
                    cs = slice(ch * 1024, (ch + 1) * 1024)
                    p_n = ppA.tile([128, 1024], F32, tag="pA",
                                   name=f"p_nlg{ch}")
                    mms(p_n, [(Vb["lhsT_nlgx"], x1),
                              (Vb["lhsT_nlgl"], lg4)], cs)
                    nc.scalar.activation(E2[:, cs], p_n[:], AF.Exp,
                                         bias=Vf["b_nlg"])
                    nc.scalar.activation(nlogsb[:, cs], p_n[:], AF.Identity,
                                         bias=Vf["b_nlg"])

                # scales column: ln(1 + e5) replicated to block rows
                p_rep = ppt.tile([128, 1], F32, tag="pt", name="p_rep")
                nc.tensor.matmul(p_rep[:], Vf["lhsT_rep5"], e5[:],
                                 start=True, stop=True)
                nc.scalar.activation(scl_col[:], p_rep[:], AF.Ln,
                                     bias=one_col[:])
                for b in range(B4):
                    nc.vector.tensor_copy(
                        L_R4[b * 32:b * 32 + K_ACT, 2 * b:2 * b + 1],
                        scl_col[b * 32:b * 32 + K_ACT, 0:1])
                    nc.vector.memset(
                        L_R4[b * 32:b * 32 + 15, 2 * b + 1:2 * b + 2], 1.0)

                # --- d3 + R into one psum tile (rows 0-15 / 32-39) -------
                dpR = pha.tile([40, CB], F32, tag="dpR")
                for ch in range(NCH):
                    cs = slice(ch * 1024, (ch + 1) * 1024)
                    p_dR = ppB.tile([40, 1024], F32, tag="pB",
                                    name=f"p_dR{ch}")
                    mms(p_dR, [(Vb["lhsT_d3"], a2)], cs, rows=slice(0, 16))
                    mms(p_dR, [(L_R4[:], E2)], cs, rows=slice(32, 40))
                    nc.scalar.activation(dpR[:, cs], p_dR[:], AF.Identity,
                                         bias=Vf["b_dR"])

                ln_sup(2)
                ln_sup(3)
                ln_sup(4)

                # --- transposes -> packed stg6 + state -------------------
                _psk.close()
                with tc.tile_pool(name="ptr", bufs=2, space="PSUM") as ptr:
                    for t in range(NTC):
                        cs = slice(t * 128, (t + 1) * 128)
                        pT = ptr.tile([128, 40], F32, tag="pT",
                                      name=f"pT{t}")
                        pTb = ptr.tile([128, 256], BF16, tag="pTb",
                                       name=f"pTb{t}")
                        nc.tensor.transpose(pT[:, 0:40], dpR[:, cs],
                                            ident[0:40, 0:40])
                        nc.tensor.transpose(pTb[:, 0:128], nz[:, cs],
                                            identb)
                        nc.tensor.transpose(pTb[:, 128:256], nlogsb[:, cs],
                                            identb)
                        nc.vector.tensor_copy(
                            stg6bv[:, :, 6 * t:6 * t + 4],
                            pT[:, 0:16].rearrange("p (b d) -> p b d", b=B4))
                        nc.vector.tensor_copy(
                            stg6bv[:, :, 6 * t + 4:6 * t + 6],
                            pT[:, 32:40].rearrange("p (b d) -> p b d", b=B4))
                        nc.vector.tensor_copy(
                            statebv[:, :, 50 * t + 2:50 * t + 34],
                            pTb[:, 0:128].rearrange("p (b f) -> p b f",
                                                    b=B4))
                        nc.vector.tensor_copy(
                            statebv[:, :, 50 * t + 34:50 * t + 49],
                            pTb[:, 128:256].rearrange("p (b f) -> p b f",
                                                      b=B4)[:, :, 0:15])

                    # ---- packed scalar chain (all [128, JT]) ------------
                    dp0v = stg6[:, 0:6 * JT:6]
                    dp1v = stg6[:, 1:6 * JT:6]
                    dp2v = stg6[:, 2:6 * JT:6]
                    dp3v = stg6[:, 3:6 * JT:6]
                    Rnv = stg6[:, 4:6 * JT:6]
                    Rdv = stg6[:, 5:6 * JT:6]
                    nhv = hl2[:, 0:2 * JT:2]
                    nlv = hl2[:, 1:2 * JT:2]

                    with tc.tile_pool(name="pk", bufs=20) as pk:
                        def pkt(name):
                            return pk.tile([128, JT], F32, tag="pk",
                                           name=name)

                        # sig_h/sig_l = softplus(dp2/3)+0.01 (exp/ln)
                        for dpv, epsv, rv, dadd, outv in (
                                (dp2v, Vf["eh_p"], Vf["rh_p"], dp0v, nhv),
                                (dp3v, Vf["el_p"], Vf["rlow_p"], dp1v, nlv)):
                            ex = pkt("ex")
                            nc.scalar.activation(ex[:], dpv, AF.Exp)
                            ex2 = pkt("ex2")
                            nc.vector.tensor_scalar_add(ex2[:], ex[:], 1.0)
                            sp = pkt("sp")
                            nc.scalar.activation(sp[:], ex2[:], AF.Ln)
                            m1 = pkt("m1")
                            nc.vector.scalar_tensor_tensor(
                                m1[:], sp[:], 0.01, epsv, ALU.add, ALU.mult)
                            s1 = pkt("s1")
                            nc.vector.tensor_tensor(s1[:], m1[:], rv,
                                                    ALU.add)
                            s2 = pkt("s2")
                            nc.vector.tensor_tensor(s2[:], s1[:], dadd,
                                                    ALU.add)
                            nc.vector.tensor_scalar_max(outv, s2[:], 0.0)

                        # R = clip(R_src * Rn/Rd, .15, 4)
                        rdr = pkt("rdr")
                        nc.vector.reciprocal(rdr[:], Rdv)
                        rr1 = pkt("rr1")
                        nc.vector.tensor_tensor(rr1[:], rdr[:], Rnv,
                                                ALU.mult)
                        Rv0 = pkt("Rv0")
                        nc.vector.tensor_scalar(Rv0[:], rr1[:],
                                                rsrc_col[:, 0:1], None,
                                                ALU.mult)
                        Rv = pkt("Rv")
                        nc.vector.tensor_scalar(Rv[:], Rv0[:], 0.15, 4.0,
                                                ALU.max, ALU.min)
                        rcpR = pkt("rcpR")
                        nc.vector.reciprocal(rcpR[:], Rv[:])
                        # zz = (obs - nh)/R ; xw = alpha*zz/sqrt(2)
                        zzt = pkt("zzt")
                        nc.vector.tensor_scalar(zzt[:], nhv,
                                                obs_col[:, 0:1], -1.0,
                                                ALU.subtract, ALU.mult)
                        zz = pkt("zz")
                        nc.vector.tensor_tensor(zz[:], zzt[:], rcpR[:],
                                                ALU.mult)
                        xw = pkt("xw")
                        nc.vector.tensor_scalar(xw[:], zz[:], asc[:, 0:1],
                                                None, ALU.mult)
                        # scalar SIG group: just the erf
                        erf_t = pkt("erf_t")
                        nc.scalar.activation(erf_t[:], xw[:], AF.Erf)
                        nd = pkt("nd")
                        nc.vector.tensor_scalar(nd[:], erf_t[:], 0.5, 0.5,
                                                ALU.mult, ALU.add)

                        # scalar NL group 3
                        lc = pkt("lc")
                        nc.scalar.activation(lc[:], nd[:], AF.Ln)
                        lnR = pkt("lnR")
                        nc.scalar.activation(lnR[:], Rv[:], AF.Ln)
                        zz2 = pkt("zz2")
                        nc.vector.tensor_tensor(zz2[:], zz[:], zz[:],
                                                ALU.mult)
                        l1 = pkt("l1")
                        nc.vector.scalar_tensor_tensor(
                            l1[:], zz2[:], -0.5, lc[:], ALU.mult, ALU.add)
                        l2 = pkt("l2")
                        nc.vector.scalar_tensor_tensor(
                            l2[:], lnR[:], -1.0, l1[:], ALU.mult, ALU.add)
                        lw = pkt("lw")
                        nc.vector.scalar_tensor_tensor(
                            lw[:], Vf["lw0_p"], C_LL, l2[:], ALU.add,
                            ALU.add)
                        nc.vector.tensor_reduce(lwm[:], lw[:], AX.X,
                                                ALU.max)
                        ptl = ptr.tile([1, 128], F32, tag="ptl",
                                       name="ptl")
                        nc.tensor.transpose(ptl[:], lwm[:], ident)
                        nc.vector.tensor_copy(lwrow[:], ptl[:])
                        nc.vector.tensor_reduce(lwm1[:], lwrow[:], AX.X,
                                                ALU.max)
                        prw = ptr.tile([128, 1], F32, tag="ptl",
                                       name="rep_lwmax")
                        nc.tensor.matmul(prw[:], ones128[:], lwm1[:],
                                         start=True, stop=True)
                        nc.vector.tensor_copy(lwmax_col[:], prw[:])
                        dsh = pkt("dsh")
                        nc.vector.tensor_scalar(dsh[:], lw[:],
                                                lwmax_col[:, 0:1], LWCLAMP,
                                                ALU.subtract, ALU.max)
                        nc.scalar.activation(w_p[:], dsh[:], AF.Exp,
                                             scale=2.0)

                    # remaining big-loop Ln's (scalar NL group 3 tail)
                    for s in range(5, SUP):
                        ln_sup(s)

                    # ---- state: nh/nl cols, ones col, weight fold -------
                    nc.vector.tensor_copy(
                        statemv[:, :, 0:2],
                        hl2[:, :].rearrange("p (m f) -> p m f", m=JT))
                    nc.vector.memset(statemv[:, :, 49:50], 1.0)
                    rsq_sup(2)
                    rsq_sup(3)
                    rsq_sup(4)
                    wb = w_p[:, :].unsqueeze(-1).to_broadcast(
                        [128, JT, 50])
                    nc.vector.tensor_tensor(statemv, statemv, wb, ALU.mult)

            # =================== big loop ================================
            with (
                tc.tile_pool(name="pyp", bufs=1, space="PSUM") as pyp,
                tc.tile_pool(name="pout", bufs=2, space="PSUM") as pout,
                tc.tile_pool(name="outp", bufs=2) as outp,
            ):
                py = pyp.tile([50, R], F32, tag="py")
                for s in range(SUP):
                    if s >= 5:
                        rsq_sup(s)
                    for k in range(G):
                        jt = s * G + k
                        lhsT = state[:, jt * 50:(jt + 1) * 50]
                        for b5 in range(R // 512):
                            rs = slice(k * R + b5 * 512,
                                       k * R + (b5 + 1) * 512)
                            ps = slice(b5 * 512, (b5 + 1) * 512)
                            nc.tensor.matmul(py[:, ps], lhsT,
                                             t_tiles[s][:, rs],
                                             start=(jt == 0),
                                             stop=(jt == JT - 1))

                # ---- output: transpose back, divide by denominator ------
                nc.vector.tensor_copy(ysb[:], py[:])
                for ob in range(OB):
                    obs_ = slice(ob * OW, (ob + 1) * OW)
                    po = pout.tile([OW, 50], F32, tag="po", name="po")
                    nc.tensor.transpose(po[:], ysb[:, obs_],
                                        ident[0:50, 0:50])
                    osb = outp.tile([OW, 50], F32, tag="osb", name="osb")
                    nc.vector.tensor_copy(osb[:], po[:])
                    rden = outp.tile([OW, 1], F32, tag="rden", name="rden")
                    nc.vector.reciprocal(rden[:], osb[:, 49:50])
                    yt = outp.tile([OW, 49], F32, tag="yt", name="yt")
                    nc.vector.tensor_scalar(yt[:], osb[:, 0:49],
                                            rden[:, 0:1], None, ALU.mult)
                    nc.sync.dma_start(d_y[obs_, :], yt[:])

        for free in reversed(_keep):
            free()

    nc.compile()
    return nc


# ---------------------------------------------------------------------------
# host-side preparation
# ---------------------------------------------------------------------------

def _f32(x):
    return np.ascontiguousarray(np.asarray(x, dtype=np.float32))


def _bf16(x):
    import ml_dtypes
    return np.ascontiguousarray(np.asarray(x).astype(ml_dtypes.bfloat16))


def prep_inputs(inputs, n_cores):
    g = {k: _f32(v) for k, v in inputs.items()}
    N = g["z"].shape[0]
    JT = N // 128
    CB = N // B4
    R = N // n_cores
    h = g["h_t"]

    def packed(a):
        return np.ascontiguousarray(a.reshape(JT, 128).T)

    W_rt1, W_d1, W_g, W_c = g["W_rt1"], g["W_d1"], g["W_g"], g["W_c"]
    b_rt1 = g["b_rt1"] + W_rt1[:, :64] @ h
    b_d1 = g["b_d1"] + W_d1[:, :64] @ h
    b_g = g["b_g"] + W_g[:, :64] @ h
    b_c = g["b_c"] + W_c[:, :64] @ h

    # block-diagonal builders for the 4-way stacked layout
    def bdiag(blk, rin_pitch, cout_pitch, rtot, ctot):
        out = np.zeros((rtot, ctot), np.float32)
        r, c = blk.shape
        for b in range(B4):
            out[b * rin_pitch:b * rin_pitch + r,
                b * cout_pitch:b * cout_pitch + c] = blk
        return out

    def bias4(vec, pitch=32, rows=128):
        out = np.zeros((rows, 1), np.float32)
        for b in range(B4):
            out[b * pitch:b * pitch + len(vec), 0] = vec
        return out

    # E1: remb_un = E @ embed[:5] ; S1 broadcast to 32 rows per block
    e1r = np.zeros((15, 32), np.float32)
    e1r[:K_ACT, 0:16] = g["embed"][:K_ACT]
    e1s = np.ones((15, 32), np.float32)
    lhsT_E1r = bdiag(e1r, 15, 32, 60, 128)
    lhsT_E1s = bdiag(e1s, 15, 32, 60, 128)

    # x1 = silu(W_rt1[:, 64:80] . remb + b)
    x1blk = np.zeros((32, 32), np.float32)
    x1blk[0:16, :] = W_rt1[:, 64:80].T
    lhsT_x1 = bdiag(x1blk, 32, 32, 128, 128)

    # nlg: new_logits = 0.3*W_rt2.x1 (first 5) + {0.7,1.0}*logits
    nlgx = np.zeros((32, 32), np.float32)
    nlgx[:, :K_ACT] = 0.3 * g["W_rt2"].T[:, :K_ACT]
    lhsT_nlgx = bdiag(nlgx, 32, 32, 128, 128)
    nlgl = np.zeros((15, 32), np.float32)
    for j in range(15):
        nlgl[j, j] = 0.7 if j < K_ACT else 1.0
    lhsT_nlgl = bdiag(nlgl, 15, 32, 60, 128)
    b_nlg = np.zeros(32, np.float32)
    b_nlg[:K_ACT] = 0.3 * g["b_rt2"][:K_ACT]

    # d1 halves: remb part (rows 0-15) and z part
    def dh(W, lo, hi, src):   # src: 64..80 remb / 80..112 z
        blk = np.zeros((32 if src == "r" else 32, 32), np.float32)
        if src == "r":
            blk = np.zeros((32, 32), np.float32)
            blk[0:16, :] = W[lo:hi, 64:80].T
        else:
            blk = W[lo:hi, 80:112].T
        return bdiag(blk, 32, 32, 128, 128)

    lhsT_d1r0 = dh(W_d1, 0, 32, "r")
    lhsT_d1z0 = dh(W_d1, 0, 32, "z")
    lhsT_d1r1 = dh(W_d1, 32, 64, "r")
    lhsT_d1z1 = dh(W_d1, 32, 64, "z")
    lhsT_d2a0 = bdiag(g["W_d2"][:, 0:32].T, 32, 32, 128, 128)
    lhsT_d2a1 = bdiag(g["W_d2"][:, 32:64].T, 32, 32, 128, 128)
    lhsT_d3 = bdiag(g["W_d3"].T, 32, 4, 128, 16)
    lhsT_gr = dh(0.5 * W_g, 0, 32, "r")
    lhsT_gz = dh(0.5 * W_g, 0, 32, "z")
    lhsT_cr = dh(W_c, 0, 32, "r")
    lhsT_cz = dh(W_c, 0, 32, "z")

    b_dR = np.zeros((40, 1), np.float32)
    for b in range(B4):
        b_dR[b * 4:b * 4 + 4, 0] = g["b_d3"]

    lhsT_rep5 = np.zeros((5, 128), np.float32)
    for b in range(B4):
        for j in range(K_ACT):
            lhsT_rep5[j, b * 32 + j] = 1.0

    lhsT_a1 = np.concatenate([g["W_a1"].T, g["b_a1"][None, :]], 0)
    h_colv = np.concatenate([h, np.ones(1, np.float32)])[:, None]

    piecesf = {
        "ident": np.eye(128, dtype=np.float32),
        "lhsT_a1": _f32(lhsT_a1), "lhsT_a2": _f32(g["W_a2"].T),
        "brow_a2": _f32(g["b_a2"][None, :]), "h_col": _f32(h_colv),
        "lhsT_rep5": lhsT_rep5,
        "log_obs5": _f32(g["log_obs_scale"][:K_ACT][:, None]),
        "logR0": _f32(g["log_R"][0].reshape(1, 1)),
        "obs11": _f32(np.asarray(g["obs_remaining"]).reshape(1, 1)),
        "b_x1": bias4(b_rt1), "b_d1h0": bias4(b_d1[0:32]),
        "b_d1h1": bias4(b_d1[32:64]), "b_d2": bias4(g["b_d2"]),
        "b_dR": b_dR, "b_nlg": bias4(b_nlg),
        "b_g": bias4(0.5 * b_g), "b_c": bias4(b_c),
        "rh_p": packed(g["remaining_high"]),
        "rlow_p": packed(g["remaining_low"]),
        "eh_p": packed(g["eps_high"]),
        "el_p": packed(g["eps_low"]),
        "lw0_p": packed(g["log_weights"]),
    }
    piecesb = {
        "identb": np.eye(128, dtype=np.float32),
        "lhsT_E1r": lhsT_E1r, "lhsT_E1s": lhsT_E1s, "lhsT_x1": lhsT_x1,
        "lhsT_nlgx": lhsT_nlgx, "lhsT_nlgl": lhsT_nlgl,
        "lhsT_d1r0": lhsT_d1r0, "lhsT_d1z0": lhsT_d1z0,
        "lhsT_d1r1": lhsT_d1r1, "lhsT_d1z1": lhsT_d1z1,
        "lhsT_d2a0": lhsT_d2a0, "lhsT_d2a1": lhsT_d2a1,
        "lhsT_d3": lhsT_d3,
        "lhsT_gr": lhsT_gr, "lhsT_gz": lhsT_gz,
        "lhsT_cr": lhsT_cr, "lhsT_cz": lhsT_cz,
    }

    import ml_dtypes

    def pack_blob(spec, pieces, dt):
        CP = sum(m for _, _, m in spec)
        blob = np.zeros((128, CP), dt)
        off = 0
        for nm, k, m in spec:
            arr = pieces[nm]
            assert arr.shape == (k, m), (nm, arr.shape, (k, m))
            blob[0:k, off:off + m] = arr.astype(dt)
            off += m
        return blob

    pf = pack_blob(_param_spec_f32(), piecesf, np.float32)
    pb = pack_blob(_param_spec_bf16(), piecesb, ml_dtypes.bfloat16)

    # 4-way stacked activations (bf16)
    z4 = np.ascontiguousarray(
        g["z"].reshape(B4, CB, 32).transpose(0, 2, 1).reshape(128, CB))
    lg4 = np.ascontiguousarray(
        g["regime_logits"].reshape(B4, CB, 15).transpose(0, 2, 1)
        .reshape(60, CB))

    common = dict(
        z4=_bf16(z4),
        lg4=_bf16(lg4),
        pf=pf,
        pb=np.ascontiguousarray(pb),
    )

    u = g["u_gumbel"]
    in_maps = []
    for c in range(n_cores):
        m = dict(common)
        m["uT"] = np.ascontiguousarray(u[c * R:(c + 1) * R, :].T)
        in_maps.append(m)
    return in_maps


_PROG_CACHE = {}
TRACE = False           # set True (e.g. from test.py) to profile on HW
LAST_EXEC_NS = None


def kernel(**inputs):
    global LAST_EXEC_NS
    n_cores = 8
    N = int(np.asarray(inputs["z"]).shape[0])
    R = N // n_cores
    key = (N, R)
    if key not in _PROG_CACHE:
        _PROG_CACHE[key] = build_program(N, R)
    nc = _PROG_CACHE[key]
    in_maps = prep_inputs(inputs, n_cores)
    res = run_bass_kernel_spmd(nc, in_maps, list(range(n_cores)),
                               trace=TRACE)
    LAST_EXEC_NS = res.exec_time_ns
    outs = [res.results[c]["y"] for c in range(n_cores)]
    return np.concatenate(outs, axis=0).astype(np.float32)


# revision 10
# speedup vs baseline: 4.1334x; 1.1479x over previous
"""Trainium2 Bass kernel for nn_DifferentiableParticleFilter (N=8192, 8 cores).

Sharding: the (N,N) soft-resample matrix is sharded by output rows (1024 per
core); the per-particle network + state (N,49) is computed replicated on each
core.  Host pre-transposes each u_gumbel shard so the contraction axis lands
on SBUF partitions.

Big-tensor pipeline per supertile (tau = 0.5 -> exp(g/tau) = 1/ln(u+eps)^2):
    DMA u (fp32) -> scalar Ln in place -> fused custom DVE op
    t = (1/L)^2 (bitnot exponent-flip seed + one tuned NR step + square,
    bf16 out) -> bf16 PE matmul against the weight-folded state.
Phase A (per-particle nets) runs in a 4-way particle-stacked layout
([4*d rows, 2048 cols]) with bf16 matmuls; biases fold into activation bias
columns; particle weights fold into the state via one broadcast TT; the
sigmoid gate is computed as (1+tanh(x/2))/2 so every phase-A transcendental
sits in the silu/tanh table set (4 ACT set switches total).
"""

import numpy as np

import concourse.bass as bass
import concourse.tile as tile
from concourse import bacc, mybir
from concourse.bass_utils import run_bass_kernel_spmd

F32 = mybir.dt.float32
BF16 = mybir.dt.bfloat16
AF = mybir.ActivationFunctionType
ALU = mybir.AluOpType
AX = mybir.AxisListType

K_ACT = 5
EPS = 1.0e-10
LWCLAMP = -30.0
C_LL = float(np.log(2.0) - 0.5 * np.log(2.0 * np.pi))
INV_SQRT2 = float(1.0 / np.sqrt(2.0))

# fused DVE op: out = s0 * ( not_x*(A - B*(x*not_x)) )^2  ~=  s0/x^2
# A,B minimax-tuned for one NR step over u = x*bitcast(~x) in [-4.5,-4].
RSQ_A = -0.47141455934487236
RSQ_B = 0.05546133703759453

B4 = 4                  # particle-stacking factor for phase A
_RSQ_OP = None


def _register_recip_sq():
    """Register the fused (1/x)^2 custom DVE op (idempotent)."""
    global _RSQ_OP
    if _RSQ_OP is not None:
        return _RSQ_OP
    import concourse.dve_ops as dve_ops
    from concourse.dve_ops import DveOp
    from concourse.dve_spec import (AluOp, Bin, C0, C1, C2, Spec, Src0,
                                    lower, _has_src1)
    from concourse.dve_uop import DveOpSpec

    name = "RECIP_SQ_W_ANT"
    for op in dve_ops.OPS:
        if op.name == name:
            _RSQ_OP = op
            return op

    _not = Bin(AluOp.BITWISE_NOT, Src0, Src0)
    _y1 = _not * (C2 - C1 * (Src0 * _not))
    _body = (_y1 * _y1) * C0

    def _ref(in0, in1, c0, c1, c2):
        not_x = (~in0.view(np.int32)).view(np.float32)
        y1 = not_x * (c2 - c1 * (in0 * not_x))
        return (y1 * y1) * c0

    spec = Spec(body=_body, reference=_ref)
    opcode = dve_ops._CUSTOM_DVE_ROW_BASE + len(dve_ops.OPS)
    shas = {}
    for ver in ("v3", "v4"):
        try:
            uops = lower(spec, ver=ver)
            shas[ver] = DveOpSpec(
                name=name, uops=uops, opcode=opcode, rd1_en=_has_src1(spec)
            ).sha(ver)
        except Exception:
            pass
    op = DveOp(name, spec, subdim=False, uops_sha=shas)
    dve_ops.OPS.append(op)
    dve_ops.CUSTOM_DVE_SPECS[name] = spec
    dve_ops._SUB_OPCODE_FOR_NAME[name] = opcode
    _RSQ_OP = op
    return op


# fp32 parameter blob (one DMA): (name, n_partitions, n_cols)
def _param_spec_f32():
    return [
        ("ident", 128, 128),
        ("lhsT_a1", 65, 16), ("lhsT_a2", 16, 1), ("brow_a2", 1, 1),
        ("h_col", 65, 1), ("lhsT_rep5", 5, 128),
        ("log_obs5", 5, 1), ("logR0", 1, 1), ("obs11", 1, 1),
        ("b_x1", 128, 1), ("b_d1h0", 128, 1), ("b_d1h1", 128, 1),
        ("b_d2", 128, 1), ("b_dR", 40, 1), ("b_nlg", 128, 1),
        ("b_g", 128, 1), ("b_c", 128, 1),
        ("rh_p", 128, 64), ("rlow_p", 128, 64), ("eh_p", 128, 64),
        ("el_p", 128, 64), ("lw0_p", 128, 64),
    ]


# bf16 parameter blob: block-diagonal lhsTs for the 4-way stacked layout
def _param_spec_bf16():
    return [
        ("identb", 128, 128),
        ("lhsT_E1r", 60, 128), ("lhsT_E1s", 60, 128),
        ("lhsT_x1", 128, 128),
        ("lhsT_nlgx", 128, 128), ("lhsT_nlgl", 60, 128),
        ("lhsT_d1r0", 128, 128), ("lhsT_d1z0", 128, 128),
        ("lhsT_d1r1", 128, 128), ("lhsT_d1z1", 128, 128),
        ("lhsT_d2a0", 128, 128), ("lhsT_d2a1", 128, 128),
        ("lhsT_d3", 128, 16),
        ("lhsT_gr", 128, 128), ("lhsT_gz", 128, 128),
        ("lhsT_cr", 128, 128), ("lhsT_cz", 128, 128),
    ]


# ---------------------------------------------------------------------------
# device program (SPMD - one program, per-core inputs differ)
# ---------------------------------------------------------------------------

def build_program(n_particles, rows_per_core):
    N = int(n_particles)
    R = int(rows_per_core)
    JT = N // 128                 # 64 j-tiles (contraction tiles of 128)
    CB = N // B4                  # stacked cols per block (2048)
    NCH = CB // 1024              # phase-A 1024-col psum chunks (2)
    G = 4                         # j-tiles per supertile
    SUP = JT // G                 # 16 supertiles
    TW = G * R                    # supertile width (4096)
    NTC = CB // 128               # transpose chunks (16)
    OW = min(128, R)
    OB = R // OW

    rsq = _register_recip_sq()

    nc = bacc.Bacc("TRN2", target_bir_lowering=False, debug=False)

    def par(name, shape, dt=F32, out=False):
        return nc.declare_dram_parameter(name, list(shape), dt, isOutput=out)

    specf = _param_spec_f32()
    CPf = sum(m for _, _, m in specf)
    specb = _param_spec_bf16()
    CPb = sum(m for _, _, m in specb)
    d_uT = par("uT", (N, R))
    d_z4 = par("z4", (128, CB), BF16)
    d_lg4 = par("lg4", (60, CB), BF16)
    d_pf = par("pf", (128, CPf))
    d_pb = par("pb", (128, CPb), BF16)
    d_y = par("y", (R, 49), out=True)

    with tile.TileContext(nc) as tc:
        _keep = []

        def sm(shape, name, dt=F32):
            t, free = tc.tile(list(shape), dt, name=name)
            _keep.append(free)
            return t

        # ---- persistent tiles -------------------------------------------
        Pf = sm((128, CPf), "Pf")
        nc.sync.dma_start(Pf[:], d_pf[:])
        Pb = sm((128, CPb), "Pb", BF16)
        nc.sync.dma_start(Pb[:], d_pb[:])

        def views(P, spec):
            v, off = {}, 0
            for nm, k, m in spec:
                v[nm] = P[0:k, off:off + m]
                off += m
            return v

        Vf = views(Pf, specf)
        Vb = views(Pb, specb)
        offs = {}
        _o = 0
        for _nm, _k, _m in specf:
            offs[_nm] = _o
            _o += _m
        ident = Vf["ident"]
        identb = Vb["identb"]

        lg4 = sm((60, CB), "lg4", BF16)
        nc.sync.dma_start(lg4[:], d_lg4[:])
        z4 = sm((128, CB), "z4", BF16)
        nc.sync.dma_start(z4[:], d_z4[:])

        state = sm((128, 50 * JT), "state", BF16)
        stg6 = sm((128, 6 * JT), "stg6")
        hl2 = sm((128, 2 * JT), "hl2")
        w_p = sm((128, JT), "w_p")
        eps_col = sm((128, 1), "eps_col")
        nc.vector.memset(eps_col[:], EPS)
        one_col = sm((128, 1), "one_col")
        nc.vector.memset(one_col[:], 1.0)
        ones128 = sm((1, 128), "ones128")
        nc.vector.memset(ones128[:], 1.0)
        L_R4 = sm((128, 8), "L_R4", BF16)
        nc.vector.memset(L_R4[:], 0.0)
        rsr = sm((1, 1), "rsr")
        rsrc_col = sm((128, 1), "rsrc_col")
        obs_col = sm((128, 1), "obs_col")
        e5 = sm((5, 1), "e5")
        scl_col = sm((128, 1), "scl_col")
        ah = sm((16, 1), "ah")
        al_sb = sm((1, 1), "al_sb")
        alpha_col = sm((128, 1), "alpha_col")
        asc = sm((128, 1), "asc")
        lwm = sm((128, 1), "lwm")
        lwmax_col = sm((128, 1), "lwmax_col")
        lwrow = sm((1, 128), "lwrow")
        lwm1 = sm((1, 1), "lwm1")
        ysb = sm((50, R), "ysb")

        statemv = state[:, :].rearrange("p (m f) -> p m f", m=JT)
        statebv = state[:, :].rearrange("p (b x) -> p b x", b=B4)
        stg6bv = stg6[:, :].rearrange("p (b x) -> p b x", b=B4)

        # ---- streaming pools (outlive phase A) --------------------------
        with (
            tc.tile_pool(name="ust", bufs=4) as ust,
            tc.tile_pool(name="tst", bufs=8) as tst,
        ):
            uT_r = d_uT.rearrange("(s k p) c -> s p k c", p=128, k=G)
            u_tiles = []
            t_tiles = []
            for s in range(SUP):
                ut = ust.tile([128, TW], F32, tag="u", name=f"u{s}")
                nc.sync.dma_start(ut.rearrange("p (k c) -> p k c", k=G),
                                  uT_r[s])
                u_tiles.append(ut)
                t_tiles.append(
                    tst.tile([128, TW], BF16, tag="t", name=f"t{s}"))

            def ln_sup(s):
                nc.scalar.activation(u_tiles[s][:], u_tiles[s][:], AF.Ln,
                                     bias=eps_col[:])

            def rsq_sup(s):
                nc.vector._custom_dve(rsq, out=t_tiles[s][:],
                                      in0=u_tiles[s][:],
                                      s0=1.0, s1=RSQ_B, imm2=RSQ_A)

            # =================== phase A =================================
            with (
                tc.tile_pool(name="pha", bufs=1) as pha,
                tc.tile_pool(name="pr2", bufs=1) as pr2,
            ):
                from contextlib import ExitStack
                _psk = ExitStack()
                ppA = _psk.enter_context(
                    tc.tile_pool(name="ppA", bufs=2, space="PSUM"))
                ppB = _psk.enter_context(
                    tc.tile_pool(name="ppB", bufs=1, space="PSUM"))
                ppt = _psk.enter_context(
                    tc.tile_pool(name="ppt", bufs=1, space="PSUM"))
                # --- scalar NL group 1 -----------------------------------
                ln_sup(0)
                ln_sup(1)
                Esb = pha.tile([60, CB], BF16, tag="Esb")
                nc.scalar.activation(Esb[:], lg4[:], AF.Exp)
                nc.scalar.activation(e5[:], Vf["log_obs5"], AF.Exp)
                nc.scalar.activation(rsr[:], Vf["logR0"], AF.Exp)

                def mms(psum_t, pairs, cs, rows=slice(0, 128)):
                    """psum_t[rows,:1024] += sum_i lhsT_i.T @ rhs_i[:, cs],
                    as 2x512-col matmuls (one PSUM bank each)."""
                    for b5 in range(2):
                        bs = slice(b5 * 512, (b5 + 1) * 512)
                        gs = slice(cs.start + b5 * 512,
                                   cs.start + (b5 + 1) * 512)
                        for i, (lt, rh) in enumerate(pairs):
                            nc.tensor.matmul(psum_t[rows, bs], lt,
                                             rh[:, gs],
                                             start=(i == 0),
                                             stop=(i == len(pairs) - 1))

                def mlp_layer(out_sb, pairs, af, bias_ap, nm):
                    for ch in range(NCH):
                        cs = slice(ch * 1024, (ch + 1) * 1024)
                        ps = ppA.tile([128, 1024], F32, tag="pA",
                                      name=f"{nm}{ch}")
                        mms(ps, pairs, cs)
                        if bias_ap is None:
                            nc.scalar.activation(out_sb[:, cs], ps[:], af)
                        else:
                            nc.scalar.activation(out_sb[:, cs], ps[:], af,
                                                 bias=bias_ap)

                # --- E1: remb = (E @ embed5) / S1 ------------------------
                remb = pha.tile([128, CB], BF16, tag="remb")
                for ch in range(NCH):
                    cs = slice(ch * 1024, (ch + 1) * 1024)
                    p_r = ppA.tile([128, 1024], F32, tag="pA",
                                   name=f"p_remb{ch}")
                    mms(p_r, [(Vb["lhsT_E1r"], Esb)], cs)
                    p_s = ppA.tile([128, 1024], F32, tag="pA",
                                   name=f"p_s1{ch}")
                    mms(p_s, [(Vb["lhsT_E1s"], Esb)], cs)
                    r2 = pr2.tile([128, 1024], F32, tag="r2",
                                  name=f"r2{ch}")
                    nc.vector.reciprocal_approx_fast(r2[:], p_s[:])
                    nc.vector.tensor_tensor(remb[:, cs], p_r[:], r2[:],
                                            ALU.mult)

                # custom op for supertiles 0,1 (frees their u buffers)
                rsq_sup(0)
                rsq_sup(1)

                # --- scalar SILU/TANH group ------------------------------
                x1 = pha.tile([128, CB], BF16, tag="x1")
                mlp_layer(x1, [(Vb["lhsT_x1"], remb)], AF.Silu,
                          Vf["b_x1"], "p_x1")
                a1h0 = pha.tile([128, CB], BF16, tag="a1h0")
                mlp_layer(a1h0, [(Vb["lhsT_d1r0"], remb),
                                 (Vb["lhsT_d1z0"], z4)], AF.Silu,
                          Vf["b_d1h0"], "p_d1a")
                a1h1 = pha.tile([128, CB], BF16, tag="a1h1")
                mlp_layer(a1h1, [(Vb["lhsT_d1r1"], remb),
                                 (Vb["lhsT_d1z1"], z4)], AF.Silu,
                          Vf["b_d1h1"], "p_d1b")
                a2 = pha.tile([128, CB], BF16, tag="a2")
                mlp_layer(a2, [(Vb["lhsT_d2a0"], a1h0),
                               (Vb["lhsT_d2a1"], a1h1)], AF.Silu,
                          Vf["b_d2"], "p_d2")
                th = pha.tile([128, CB], BF16, tag="a1h0", name="th")
                mlp_layer(th, [(Vb["lhsT_gr"], remb),
                               (Vb["lhsT_gz"], z4)], AF.Tanh,
                          Vf["b_g"], "p_g")
                cand = pha.tile([128, CB], BF16, tag="a1h1", name="cand")
                mlp_layer(cand, [(Vb["lhsT_cr"], remb),
                                 (Vb["lhsT_cz"], z4)], AF.Tanh,
                          Vf["b_c"], "p_c")

                # --- alpha (scalar path, silu group) ---------------------
                pa1 = ppt.tile([16, 1], F32, tag="pt", name="pa1")
                nc.tensor.matmul(pa1[:], Vf["lhsT_a1"], Vf["h_col"],
                                 start=True, stop=True)
                nc.scalar.activation(ah[:], pa1[:], AF.Silu)
                pal = ppt.tile([1, 1], F32, tag="pt", name="pal")
                nc.tensor.matmul(pal[:], Vf["lhsT_a2"], ah[:],
                                 start=True, stop=False)
                nc.tensor.matmul(pal[:], Vf["brow_a2"],
                                 one_col[0:1, 0:1], start=False, stop=True)
                nc.vector.tensor_copy(al_sb[:], pal[:])

                def replicate_col(dst_col, src11, nm):
                    pr = ppt.tile([128, 1], F32, tag="pt", name="rep_" + nm)
                    nc.tensor.matmul(pr[:], ones128[:], src11, start=True,
                                     stop=True)
                    nc.vector.tensor_copy(dst_col[:], pr[:])

                replicate_col(alpha_col, al_sb[:], "alpha")
                nc.vector.tensor_scalar_mul(asc[:], alpha_col[:], INV_SQRT2)
                replicate_col(obs_col, Vf["obs11"], "obs")
                nc.vector.tensor_scalar(rsr[:], rsr[:], 0.15, 2.5,
                                        ALU.max, ALU.min)
                replicate_col(rsrc_col, rsr[:], "rsrc")

                # --- nz on gpsimd: nz = cand + 0.5*(1+th)*(z-cand) -------
                nzt = pha.tile([128, CB], BF16, tag="Esb", name="nzt")
                nc.gpsimd.tensor_tensor(nzt[:], z4[:], cand[:], ALU.subtract)
                nc.vector.scalar_tensor_tensor(nzt[:], th[:], 1.0, nzt[:],
                                               ALU.add, ALU.mult)
                nz = pha.tile([128, CB], BF16, tag="remb", name="nz")
                nc.vector.scalar_tensor_tensor(nz[:], nzt[:], 0.5, cand[:],
                                               ALU.mult, ALU.add)

                # --- nlg -> E2 / new_logits (scalar NL group 2) ----------
                E2 = pha.tile([128, CB], BF16, tag="x1", name="E2")
                nlogsb = pha.tile([128, CB], BF16, tag="nlogsb")
                for ch in range(NCH):
                    cs = slice(ch * 1024, (ch + 1) * 1024)
                    p_n = ppA.tile([128, 1024], F32, tag="pA",
                                   name=f"p_nlg{ch}")
                    mms(p_n, [(Vb["lhsT_nlgx"], x1),
                              (Vb["lhsT_nlgl"], lg4)], cs)
                    nc.scalar.activation(E2[:, cs], p_n[:], AF.Exp,
                                         bias=Vf["b_nlg"])
                    nc.scalar.activation(nlogsb[:, cs], p_n[:], AF.Identity,
                                         bias=Vf["b_nlg"])

                # scales column: ln(1 + e5) replicated to block rows
                p_rep = ppt.tile([128, 1], F32, tag="pt", name="p_rep")
                nc.tensor.matmul(p_rep[:], Vf["lhsT_rep5"], e5[:],
                                 start=True, stop=True)
                nc.scalar.activation(scl_col[:], p_rep[:], AF.Ln,
                                     bias=one_col[:])
                for b in range(B4):
                    nc.vector.tensor_copy(
                        L_R4[b * 32:b * 32 + K_ACT, 2 * b:2 * b + 1],
                        scl_col[b * 32:b * 32 + K_ACT, 0:1])
                    nc.vector.memset(
                        L_R4[b * 32:b * 32 + 15, 2 * b + 1:2 * b + 2], 1.0)

                # --- d3 + R into one psum tile (rows 0-15 / 32-39) -------
                dpR = pha.tile([40, CB], F32, tag="dpR")
                for ch in range(NCH):
                    cs = slice(ch * 1024, (ch + 1) * 1024)
                    p_dR = ppB.tile([40, 1024], F32, tag="pB",
                                    name=f"p_dR{ch}")
                    mms(p_dR, [(Vb["lhsT_d3"], a2)], cs, rows=slice(0, 16))
                    mms(p_dR, [(L_R4[:], E2)], cs, rows=slice(32, 40))
                    nc.scalar.activation(dpR[:, cs], p_dR[:], AF.Identity,
                                         bias=Vf["b_dR"])

                ln_sup(2)
                ln_sup(3)
                ln_sup(4)

                # --- transposes -> packed stg6 + state -------------------
                _psk.close()
                with tc.tile_pool(name="ptr", bufs=2, space="PSUM") as ptr:
                    for t in range(NTC):
                        cs = slice(t * 128, (t + 1) * 128)
                        pT = ptr.tile([128, 40], F32, tag="pT",
                                      name=f"pT{t}")
                        pTb = ptr.tile([128, 256], BF16, tag="pTb",
                                       name=f"pTb{t}")
                        nc.tensor.transpose(pT[:, 0:40], dpR[:, cs],
                                            ident[0:40, 0:40])
                        nc.tensor.transpose(pTb[:, 0:128], nz[:, cs],
                                            identb)
                        nc.tensor.transpose(pTb[:, 128:256], nlogsb[:, cs],
                                            identb)
                        nc.vector.tensor_copy(
                            stg6bv[:, :, 6 * t:6 * t + 4],
                            pT[:, 0:16].rearrange("p (b d) -> p b d", b=B4))
                        nc.vector.tensor_copy(
                            stg6bv[:, :, 6 * t + 4:6 * t + 6],
                            pT[:, 32:40].rearrange("p (b d) -> p b d", b=B4))
                        nc.vector.tensor_copy(
                            statebv[:, :, 50 * t + 2:50 * t + 34],
                            pTb[:, 0:128].rearrange("p (b f) -> p b f",
                                                    b=B4))
                        nc.vector.tensor_copy(
                            statebv[:, :, 50 * t + 34:50 * t + 49],
                            pTb[:, 128:256].rearrange("p (b f) -> p b f",
                                                      b=B4)[:, :, 0:15])

                    rsq_sup(2)
                    rsq_sup(3)
                    rsq_sup(4)

                    # ---- packed scalar chain (all [128, JT]) ------------
                    dp0v = stg6[:, 0:6 * JT:6]
                    dp1v = stg6[:, 1:6 * JT:6]
                    dp2v = stg6[:, 2:6 * JT:6]
                    dp3v = stg6[:, 3:6 * JT:6]
                    Rnv = stg6[:, 4:6 * JT:6]
                    Rdv = stg6[:, 5:6 * JT:6]
                    nhv = hl2[:, 0:2 * JT:2]
                    nlv = hl2[:, 1:2 * JT:2]

                    with tc.tile_pool(name="pk", bufs=12) as pk:
                        def pkt(name):
                            return pk.tile([128, JT], F32, tag="pk",
                                           name=name)

                        # sig_h/l = softplus(dp2/3)+0.01, h/l paired
                        stg6j = stg6[:, :].rearrange("p (m j) -> p m j",
                                                     j=6)
                        hl2j = hl2[:, :].rearrange("p (m j) -> p m j", j=2)
                        rhrl = Pf[0:128, offs["rh_p"]:offs["rh_p"] + 128] \
                            .rearrange("p (j m) -> p m j", j=2)
                        ehel = Pf[0:128, offs["eh_p"]:offs["eh_p"] + 128] \
                            .rearrange("p (j m) -> p m j", j=2)

                        def pk2(name):
                            t = pk.tile([128, 2 * JT], F32, tag="pk2",
                                        bufs=6, name=name)
                            return t, t[:, :].rearrange(
                                "p (m j) -> p m j", j=2)

                        ex, exj = pk2("ex")
                        nc.scalar.activation(exj, stg6j[:, :, 2:4], AF.Exp)
                        ex2, _ = pk2("ex2")
                        nc.vector.tensor_scalar_add(ex2[:], ex[:], 1.0)
                        sp, spj = pk2("sp")
                        nc.scalar.activation(sp[:], ex2[:], AF.Ln)
                        m1, m1j = pk2("m1")
                        nc.vector.scalar_tensor_tensor(
                            m1j, spj, 0.01, ehel, ALU.add, ALU.mult)
                        s1, s1j = pk2("s1")
                        nc.vector.tensor_tensor(s1j, m1j, rhrl, ALU.add)
                        s2, s2j = pk2("s2")
                        nc.vector.tensor_tensor(s2j, s1j, stg6j[:, :, 0:2],
                                                ALU.add)
                        nc.vector.tensor_scalar_max(hl2[:], s2[:], 0.0)
                        ln_sup(5)

                        # R = clip(R_src * Rn/Rd, .15, 4)
                        rdr = pkt("rdr")
                        nc.vector.reciprocal(rdr[:], Rdv)
                        rr1 = pkt("rr1")
                        nc.vector.tensor_tensor(rr1[:], rdr[:], Rnv,
                                                ALU.mult)
                        Rv0 = pkt("Rv0")
                        nc.vector.tensor_scalar(Rv0[:], rr1[:],
                                                rsrc_col[:, 0:1], None,
                                                ALU.mult)
                        Rv = pkt("Rv")
                        nc.vector.tensor_scalar(Rv[:], Rv0[:], 0.15, 4.0,
                                                ALU.max, ALU.min)
                        rcpR = pkt("rcpR")
                        nc.vector.reciprocal(rcpR[:], Rv[:])
                        # zz = (obs - nh)/R ; xw = alpha*zz/sqrt(2)
                        zzt = pkt("zzt")
                        nc.vector.tensor_scalar(zzt[:], nhv,
                                                obs_col[:, 0:1], -1.0,
                                                ALU.subtract, ALU.mult)
                        zz = pkt("zz")
                        nc.vector.tensor_tensor(zz[:], zzt[:], rcpR[:],
                                                ALU.mult)
                        xw = pkt("xw")
                        nc.vector.tensor_scalar(xw[:], zz[:], asc[:, 0:1],
                                                None, ALU.mult)
                        # scalar SIG group: just the erf
                        erf_t = pkt("erf_t")
                        nc.scalar.activation(erf_t[:], xw[:], AF.Erf)
                        ln_sup(6)
                        nd = pkt("nd")
                        nc.vector.tensor_scalar(nd[:], erf_t[:], 0.5, 0.5,
                                                ALU.mult, ALU.add)

                        # scalar NL group 3
                        lc = pkt("lc")
                        nc.scalar.activation(lc[:], nd[:], AF.Ln)
                        lnR = pkt("lnR")
                        nc.scalar.activation(lnR[:], Rv[:], AF.Ln)
                        ln_sup(7)
                        zz2 = pkt("zz2")
                        nc.vector.tensor_tensor(zz2[:], zz[:], zz[:],
                                                ALU.mult)
                        l1 = pkt("l1")
                        nc.vector.scalar_tensor_tensor(
                            l1[:], zz2[:], -0.5, lc[:], ALU.mult, ALU.add)
                        l2 = pkt("l2")
                        nc.vector.scalar_tensor_tensor(
                            l2[:], lnR[:], -1.0, l1[:], ALU.mult, ALU.add)
                        lw = pkt("lw")
                        nc.vector.scalar_tensor_tensor(
                            lw[:], Vf["lw0_p"], C_LL, l2[:], ALU.add,
                            ALU.add)
                        nc.vector.tensor_reduce(lwm[:], lw[:], AX.X,
                                                ALU.max)
                        ptl = ptr.tile([1, 128], F32, tag="ptl",
                                       name="ptl")
                        nc.tensor.transpose(ptl[:], lwm[:], ident)
                        nc.vector.tensor_copy(lwrow[:], ptl[:])
                        nc.vector.tensor_reduce(lwm1[:], lwrow[:], AX.X,
                                                ALU.max)
                        prw = ptr.tile([128, 1], F32, tag="ptl",
                                       name="rep_lwmax")
                        nc.tensor.matmul(prw[:], ones128[:], lwm1[:],
                                         start=True, stop=True)
                        nc.vector.tensor_copy(lwmax_col[:], prw[:])
                        dsh = pkt("dsh")
                        nc.vector.tensor_scalar(dsh[:], lw[:],
                                                lwmax_col[:, 0:1], LWCLAMP,
                                                ALU.subtract, ALU.max)
                        nc.scalar.activation(w_p[:], dsh[:], AF.Exp,
                                             scale=2.0)

                    # remaining big-loop Ln's (scalar NL group 3 tail)
                    for s in range(8, SUP):
                        ln_sup(s)

                    # ---- state: nh/nl cols, ones col, weight fold -------
                    nc.vector.tensor_copy(
                        statemv[:, :, 0:2],
                        hl2[:, :].rearrange("p (m f) -> p m f", m=JT))
                    nc.vector.memset(statemv[:, :, 49:50], 1.0)
                    rsq_sup(5)
                    rsq_sup(6)
                    rsq_sup(7)
                    wb = w_p[:, :].unsqueeze(-1).to_broadcast(
                        [128, JT, 50])
                    nc.vector.tensor_tensor(statemv, statemv, wb, ALU.mult)

            # =================== big loop ================================
            with (
                tc.tile_pool(name="pyp", bufs=1, space="PSUM") as pyp,
                tc.tile_pool(name="pout", bufs=2, space="PSUM") as pout,
                tc.tile_pool(name="outp", bufs=2) as outp,
            ):
                py = pyp.tile([50, R], F32, tag="py")
                for s in range(SUP):
                    if s >= 8:
                        rsq_sup(s)
                    for k in range(G):
                        jt = s * G + k
                        lhsT = state[:, jt * 50:(jt + 1) * 50]
                        for b5 in range(R // 512):
                            rs = slice(k * R + b5 * 512,
                                       k * R + (b5 + 1) * 512)
                            ps = slice(b5 * 512, (b5 + 1) * 512)
                            nc.tensor.matmul(py[:, ps], lhsT,
                                             t_tiles[s][:, rs],
                                             start=(jt == 0),
                                             stop=(jt == JT - 1))

                # ---- output: transpose back, divide by denominator ------
                nc.vector.tensor_copy(ysb[:], py[:])
                for ob in range(OB):
                    obs_ = slice(ob * OW, (ob + 1) * OW)
                    po = pout.tile([OW, 50], F32, tag="po", name="po")
                    nc.tensor.transpose(po[:], ysb[:, obs_],
                                        ident[0:50, 0:50])
                    osb = outp.tile([OW, 50], F32, tag="osb", name="osb")
                    nc.vector.tensor_copy(osb[:], po[:])
                    rden = outp.tile([OW, 1], F32, tag="rden", name="rden")
                    nc.vector.reciprocal(rden[:], osb[:, 49:50])
                    yt = outp.tile([OW, 49], F32, tag="yt", name="yt")
                    nc.vector.tensor_scalar(yt[:], osb[:, 0:49],
                                            rden[:, 0:1], None, ALU.mult)
                    nc.sync.dma_start(d_y[obs_, :], yt[:])

        for free in reversed(_keep):
            free()

    nc.compile()
    return nc


# ---------------------------------------------------------------------------
# host-side preparation
# ---------------------------------------------------------------------------

def _f32(x):
    return np.ascontiguousarray(np.asarray(x, dtype=np.float32))


def _bf16(x):
    import ml_dtypes
    return np.ascontiguousarray(np.asarray(x).astype(ml_dtypes.bfloat16))


def prep_inputs(inputs, n_cores):
    g = {k: _f32(v) for k, v in inputs.items()}
    N = g["z"].shape[0]
    JT = N // 128
    CB = N // B4
    R = N // n_cores
    h = g["h_t"]

    def packed(a):
        return np.ascontiguousarray(a.reshape(JT, 128).T)

    W_rt1, W_d1, W_g, W_c = g["W_rt1"], g["W_d1"], g["W_g"], g["W_c"]
    b_rt1 = g["b_rt1"] + W_rt1[:, :64] @ h
    b_d1 = g["b_d1"] + W_d1[:, :64] @ h
    b_g = g["b_g"] + W_g[:, :64] @ h
    b_c = g["b_c"] + W_c[:, :64] @ h

    # block-diagonal builders for the 4-way stacked layout
    def bdiag(blk, rin_pitch, cout_pitch, rtot, ctot):
        out = np.zeros((rtot, ctot), np.float32)
        r, c = blk.shape
        for b in range(B4):
            out[b * rin_pitch:b * rin_pitch + r,
                b * cout_pitch:b * cout_pitch + c] = blk
        return out

    def bias4(vec, pitch=32, rows=128):
        out = np.zeros((rows, 1), np.float32)
        for b in range(B4):
            out[b * pitch:b * pitch + len(vec), 0] = vec
        return out

    # E1: remb_un = E @ embed[:5] ; S1 broadcast to 32 rows per block
    e1r = np.zeros((15, 32), np.float32)
    e1r[:K_ACT, 0:16] = g["embed"][:K_ACT]
    e1s = np.ones((15, 32), np.float32)
    lhsT_E1r = bdiag(e1r, 15, 32, 60, 128)
    lhsT_E1s = bdiag(e1s, 15, 32, 60, 128)

    # x1 = silu(W_rt1[:, 64:80] . remb + b)
    x1blk = np.zeros((32, 32), np.float32)
    x1blk[0:16, :] = W_rt1[:, 64:80].T
    lhsT_x1 = bdiag(x1blk, 32, 32, 128, 128)

    # nlg: new_logits = 0.3*W_rt2.x1 (first 5) + {0.7,1.0}*logits
    nlgx = np.zeros((32, 32), np.float32)
    nlgx[:, :K_ACT] = 0.3 * g["W_rt2"].T[:, :K_ACT]
    lhsT_nlgx = bdiag(nlgx, 32, 32, 128, 128)
    nlgl = np.zeros((15, 32), np.float32)
    for j in range(15):
        nlgl[j, j] = 0.7 if j < K_ACT else 1.0
    lhsT_nlgl = bdiag(nlgl, 15, 32, 60, 128)
    b_nlg = np.zeros(32, np.float32)
    b_nlg[:K_ACT] = 0.3 * g["b_rt2"][:K_ACT]

    # d1 halves: remb part (rows 0-15) and z part
    def dh(W, lo, hi, src):   # src: 64..80 remb / 80..112 z
        blk = np.zeros((32 if src == "r" else 32, 32), np.float32)
        if src == "r":
            blk = np.zeros((32, 32), np.float32)
            blk[0:16, :] = W[lo:hi, 64:80].T
        else:
            blk = W[lo:hi, 80:112].T
        return bdiag(blk, 32, 32, 128, 128)

    lhsT_d1r0 = dh(W_d1, 0, 32, "r")
    lhsT_d1z0 = dh(W_d1, 0, 32, "z")
    lhsT_d1r1 = dh(W_d1, 32, 64, "r")
    lhsT_d1z1 = dh(W_d1, 32, 64, "z")
    lhsT_d2a0 = bdiag(g["W_d2"][:, 0:32].T, 32, 32, 128, 128)
    lhsT_d2a1 = bdiag(g["W_d2"][:, 32:64].T, 32, 32, 128, 128)
    lhsT_d3 = bdiag(g["W_d3"].T, 32, 4, 128, 16)
    lhsT_gr = dh(0.5 * W_g, 0, 32, "r")
    lhsT_gz = dh(0.5 * W_g, 0, 32, "z")
    lhsT_cr = dh(W_c, 0, 32, "r")
    lhsT_cz = dh(W_c, 0, 32, "z")

    b_dR = np.zeros((40, 1), np.float32)
    for b in range(B4):
        b_dR[b * 4:b * 4 + 4, 0] = g["b_d3"]

    lhsT_rep5 = np.zeros((5, 128), np.float32)
    for b in range(B4):
        for j in range(K_ACT):
            lhsT_rep5[j, b * 32 + j] = 1.0

    lhsT_a1 = np.concatenate([g["W_a1"].T, g["b_a1"][None, :]], 0)
    h_colv = np.concatenate([h, np.ones(1, np.float32)])[:, None]

    piecesf = {
        "ident": np.eye(128, dtype=np.float32),
        "lhsT_a1": _f32(lhsT_a1), "lhsT_a2": _f32(g["W_a2"].T),
        "brow_a2": _f32(g["b_a2"][None, :]), "h_col": _f32(h_colv),
        "lhsT_rep5": lhsT_rep5,
        "log_obs5": _f32(g["log_obs_scale"][:K_ACT][:, None]),
        "logR0": _f32(g["log_R"][0].reshape(1, 1)),
        "obs11": _f32(np.asarray(g["obs_remaining"]).reshape(1, 1)),
        "b_x1": bias4(b_rt1), "b_d1h0": bias4(b_d1[0:32]),
        "b_d1h1": bias4(b_d1[32:64]), "b_d2": bias4(g["b_d2"]),
        "b_dR": b_dR, "b_nlg": bias4(b_nlg),
        "b_g": bias4(0.5 * b_g), "b_c": bias4(b_c),
        "rh_p": packed(g["remaining_high"]),
        "rlow_p": packed(g["remaining_low"]),
        "eh_p": packed(g["eps_high"]),
        "el_p": packed(g["eps_low"]),
        "lw0_p": packed(g["log_weights"]),
    }
    piecesb = {
        "identb": np.eye(128, dtype=np.float32),
        "lhsT_E1r": lhsT_E1r, "lhsT_E1s": lhsT_E1s, "lhsT_x1": lhsT_x1,
        "lhsT_nlgx": lhsT_nlgx, "lhsT_nlgl": lhsT_nlgl,
        "lhsT_d1r0": lhsT_d1r0, "lhsT_d1z0": lhsT_d1z0,
        "lhsT_d1r1": lhsT_d1r1, "lhsT_d1z1": lhsT_d1z1,
        "lhsT_d2a0": lhsT_d2a0, "lhsT_d2a1": lhsT_d2a1,
        "lhsT_d3": lhsT_d3,
        "lhsT_gr": lhsT_gr, "lhsT_gz": lhsT_gz,
        "lhsT_cr": lhsT_cr, "lhsT_cz": lhsT_cz,
    }

    import ml_dtypes

    def pack_blob(spec, pieces, dt):
        CP = sum(m for _, _, m in spec)
        blob = np.zeros((128, CP), dt)
        off = 0
        for nm, k, m in spec:
            arr = pieces[nm]
            assert arr.shape == (k, m), (nm, arr.shape, (k, m))
            blob[0:k, off:off + m] = arr.astype(dt)
            off += m
        return blob

    pf = pack_blob(_param_spec_f32(), piecesf, np.float32)
    pb = pack_blob(_param_spec_bf16(), piecesb, ml_dtypes.bfloat16)

    # 4-way stacked activations (bf16)
    z4 = np.ascontiguousarray(
        g["z"].reshape(B4, CB, 32).transpose(0, 2, 1).reshape(128, CB))
    lg4 = np.ascontiguousarray(
        g["regime_logits"].reshape(B4, CB, 15).transpose(0, 2, 1)
        .reshape(60, CB))

    common = dict(
        z4=_bf16(z4),
        lg4=_bf16(lg4),
        pf=pf,
        pb=np.ascontiguousarray(pb),
    )

    u = g["u_gumbel"]
    in_maps = []
    for c in range(n_cores):
        m = dict(common)
        m["uT"] = np.ascontiguousarray(u[c * R:(c + 1) * R, :].T)
        in_maps.append(m)
    return in_maps


_PROG_CACHE = {}
TRACE = False           # set True (e.g. from test.py) to profile on HW
LAST_EXEC_NS = None


def kernel(**inputs):
    global LAST_EXEC_NS
    n_cores = 8
    N = int(np.asarray(inputs["z"]).shape[0])
    R = N // n_cores
    key = (N, R)
    if key not in _PROG_CACHE:
        _PROG_CACHE[key] = build_program(N, R)
    nc = _PROG_CACHE[key]
    in_maps = prep_inputs(inputs, n_cores)
    res = run_bass_kernel_spmd(nc, in_maps, list(range(n_cores)),
                               trace=TRACE)
    LAST_EXEC_NS = res.exec_time_ns
    outs = [res.results[c]["y"] for c in range(n_cores)]
    return np.concatenate(outs, axis=0).astype(np.float32)
